# revision 5
# baseline (speedup 1.0000x reference)
"""LocalWindowTransformer Trainium2 kernel, v3.

Data-parallel over batch (B=8 -> 8 cores). bf16 datapath with fp8
(e4m3 DoubleRow) QKV projections: K=1024 contractions run as 4
pair-packed K=256 matmuls at 2x PE throughput (weights pre-scaled to
e4m3 range on host, dequant folded into the PSUM-drain activations).
Attention is computed in transposed-score form: scores land
[keys, queries] (one matmul per 128-key tile over its 159-query band,
window=32), exp'd and band-masked, and the AV matmul consumes them
directly; the AV rhs carries a ones column so the softmax denominator
falls out of the same matmul, and a per-tile reciprocal+scale
normalizes before one transpose back into head-major oT. LN row stats
accumulate via ones-column matmuls interleaved with the GEMM streams;
the row math (mean/var/rsqrt) runs on broadcast [128,SC] tiles because
single-partition DVE ops are ~6x slower. Weight pools are deep enough
(w1p=6, w2/wo multi-buf) to keep the PE from draining its DVFS ramp.
PSUM accumulation and LN row stats stay f32.
"""

import numpy as np

import concourse.bass as bass
import concourse.tile as tile
from concourse import mybir
from concourse.bass_utils import run_bass_kernel_spmd
from concourse.vector_clock import ScopedClock

F32 = mybir.dt.float32
BF16 = mybir.dt.bfloat16
FP8 = mybir.dt.float8e4
DR = mybir.MatmulPerfMode.DoubleRow
AF = mybir.ActivationFunctionType
OP = mybir.AluOpType

B, S, D = 8, 1024, 1024
H, HD, L, FF, NOUT, WIN = 8, 128, 4, 4096, 512, 32
ND = D // 128     # 8 feature tiles
NT = S // 128     # 8 token tiles
NF = FF // 128    # 32 ffn tiles
KT2 = ND // 2     # 4 feature-pair tiles (fp8 DoubleRow)
SC = 512          # s-chunk
NSC = S // SC     # 2
NQB = 159         # banded query cols per key tile (128 + WIN - 1)
SCALE = 1.0 / float(np.sqrt(HD))
EPS = 1e-5
N_CORES = 8


class SafeTileContext(tile.TileContext):
    """walrus in this image rejects a Drain carrying several sem waits
    ("Too many sync wait commands"). Absorb the outstanding waits into
    single-wait SP nops before the tail drain so the drain itself is
    wait-free."""

    def _drain_and_barrier(self, tick_clock, wait_clock):
        gclock = tick_clock.global_clock
        for proc in range(len(gclock)):
            tick = gclock[proc]
            if tick > 0:
                partial = ScopedClock()
                partial.require_at_least(None, proc, tick)
                nop = self.nc.sync.nop(nofuse=True)
                wait_clock.add_sem_waits(nop.ins, partial)
        self.nc.sync.drain()
        self.nc.all_engine_barrier()
        popped = self.nc._tile_sem_poison_stack.pop()
        assert popped is self._sem_poison
        self.nc.clear_and_free_semaphores(list(self.sems.allocated().values()))
        self.nc.all_engine_barrier()


def _split_multi_waits(nc):
    """This image's walrus accepts at most ONE sem wait per instruction.
    Hoist extra waits onto same-engine NoOps immediately preceding the
    instruction."""
    n = 0
    for f in nc.m.functions:
        for bb in f.blocks:
            insts = bb.instructions
            out = []
            for inst in insts:
                si = inst.sync_info
                waits = list(si.on_wait) if si is not None else []
                if len(waits) > 1:
                    for w in waits[:-1]:
                        n += 1
                        nop = mybir.InstNoOp(
                            name=f"{inst.name}-wsplit{n}",
                            engine=inst.engine,
                            ins=[], outs=[],
                            sync_info=mybir.SyncInfo(on_wait=[w], on_update=[]),
                        )
                        out.append(nop)
                    inst.sync_info = mybir.SyncInfo(
                        on_wait=[waits[-1]], on_update=list(si.on_update)
                    )
                out.append(inst)
            insts[:] = out
    return n


def build_program(skip_affine=False, taps=None):
    nc = bass.Bass()

    xt16 = nc.dram_tensor("xt16", [D, S], BF16, kind="ExternalInput")
    xt8 = nc.dram_tensor("xt8", [128, ND, S], FP8, kind="ExternalInput")
    band = nc.dram_tensor("band", [128, 160], BF16, kind="ExternalInput")
    ident = nc.dram_tensor("ident", [128, 128], BF16, kind="ExternalInput")
    wqk8 = nc.dram_tensor("wqk8", [L, 2, H, 128, KT2, 2, 128], FP8,
                          kind="ExternalInput")
    wv8 = nc.dram_tensor("wv8", [L, NSC, 128, KT2, 2, SC], FP8,
                         kind="ExternalInput")
    wot = nc.dram_tensor("wot", [L, ND, 128, ND, 128], BF16, kind="ExternalInput")
    w1t = nc.dram_tensor("w1t", [L, NF, 128, ND, 128], BF16, kind="ExternalInput")
    w2t = nc.dram_tensor("w2t", [L, ND, 128, NF, 128], BF16, kind="ExternalInput")
    wht = nc.dram_tensor("wht", [128, ND, NOUT], BF16, kind="ExternalInput")
    bqs = nc.dram_tensor("bqs", [L, 128, H], F32, kind="ExternalInput")  # bq*SCALE
    bkc = nc.dram_tensor("bkc", [L, 128, H], F32, kind="ExternalInput")
    bvb = nc.dram_tensor("bvb", [L, 1, D], BF16, kind="ExternalInput")
    dqqv = nc.dram_tensor("dqqv", [L, 128, 3], F32, kind="ExternalInput")
    boc = nc.dram_tensor("boc", [L, 128, ND], F32, kind="ExternalInput")
    b1c = nc.dram_tensor("b1c", [L, 128, NF], F32, kind="ExternalInput")
    b2c = nc.dram_tensor("b2c", [L, 128, ND], F32, kind="ExternalInput")
    g1c = nc.dram_tensor("g1c", [L, 128, ND], F32, kind="ExternalInput")
    h1c = nc.dram_tensor("h1c", [L, 128, ND], F32, kind="ExternalInput")
    g2c = nc.dram_tensor("g2c", [L, 128, ND], F32, kind="ExternalInput")
    h2c = nc.dram_tensor("h2c", [L, 128, ND], F32, kind="ExternalInput")
    onesc = nc.dram_tensor("onesc", [128, 1], BF16, kind="ExternalInput")
    onesr = nc.dram_tensor("onesr", [1, 128], BF16, kind="ExternalInput")
    bhb = nc.dram_tensor("bhb", [1, NOUT], F32, kind="ExternalInput")
    y = nc.dram_tensor("out", [S, NOUT], F32, kind="ExternalOutput")

    with SafeTileContext(nc) as tc:
        from contextlib import ExitStack

        with ExitStack() as ctx:
            ep = ctx.enter_context
            p_mm = ep(tc.tile_pool(name="p_mm", bufs=3, space="PSUM"))
            wpool = ep(tc.tile_pool(name="wpool", bufs=2))
            wsm = ep(tc.tile_pool(name="wsm", bufs=2))
            w1p = ep(tc.tile_pool(name="w1p", bufs=6))
            big = ep(tc.tile_pool(name="big", bufs=1))
            hpool = ep(tc.tile_pool(name="hpool", bufs=1))
            qkp = ep(tc.tile_pool(name="qkp", bufs=3))
            smp = ep(tc.tile_pool(name="smp", bufs=20))
            cst = ep(tc.tile_pool(name="cst", bufs=1))
            bias = ep(tc.tile_pool(name="bias", bufs=2))
            lnp = ep(tc.tile_pool(name="lnp", bufs=1))
            roll = ep(tc.tile_pool(name="roll", bufs=2))

            # ---- input + layer-0 weights first so compute starts ASAP
            x8 = big.tile([128, ND, S], FP8, tag="x8")
            for kt in range(KT2):
                nc.sync.dma_start(
                    out=x8[:, 2 * kt:2 * kt + 2, :],
                    in_=xt8[:, 2 * kt:2 * kt + 2, :],
                )

            biases = {}

            def load_biases(l):
                t = {}
                for nm, src, shape, dt in (
                    ("bq", bqs, [128, H], F32), ("bk", bkc, [128, H], F32),
                    ("bo", boc, [128, ND], F32), ("b1", b1c, [128, NF], F32),
                    ("b2", b2c, [128, ND], F32), ("g1", g1c, [128, ND], F32),
                    ("h1", h1c, [128, ND], F32), ("g2", g2c, [128, ND], F32),
                    ("h2", h2c, [128, ND], F32),
                ):
                    tl = bias.tile(shape, dt, tag=nm, name=f"{nm}_{l}")
                    nc.sync.dma_start(out=tl, in_=src[l])
                    t[nm] = tl
                dq = bias.tile([128, 3], F32, tag="dq", name=f"dq_{l}")
                nc.sync.dma_start(out=dq, in_=dqqv[l])
                t["dqq"], t["dqk"], t["dqv"] = dq[:, 0:1], dq[:, 1:2], dq[:, 2:3]
                bvt = bias.tile([128, ND, 128], BF16, tag="bv", name=f"bv_{l}")
                nc.sync.dma_start(
                    out=bvt, in_=bvb[l].to_broadcast([128, D])
                )
                t["bv"] = bvt
                biases[l] = t

            load_biases(0)

            vtiles = {}

            def load_wv(l):
                wv = []
                for dc in range(NSC):
                    wt = wpool.tile([128, KT2, 2, SC], FP8, tag="wv",
                                    name=f"wv{l}_{dc}")
                    nc.sync.dma_start(out=wt, in_=wv8[l, dc])
                    wv.append(wt)
                vtiles[l] = wv

            load_wv(0)

            # ---- constants (after the hot-path DMAs)
            bandt = cst.tile([128, 160], BF16, tag="bandt")
            nc.sync.dma_start(out=bandt, in_=band[:, :])
            idt = cst.tile([128, 128], BF16, tag="idt")
            nc.sync.dma_start(out=idt, in_=ident[:, :])
            ones_col = cst.tile([128, 1], BF16, tag="onc")
            nc.sync.dma_start(out=ones_col, in_=onesc[:, :])
            ones_row = cst.tile([1, 128], BF16, tag="onr")
            nc.sync.dma_start(out=ones_row, in_=onesr[:, :])
            bhbt = cst.tile([128, NOUT], F32, tag="bhb")
            nc.sync.dma_start(out=bhbt, in_=bhb[0:1, :].to_broadcast([128, NOUT]))
            epst = cst.tile([128, 1], F32, tag="eps")
            nc.vector.memset(epst, EPS)

            xT = big.tile([128, ND, S], BF16, tag="xT")
            for di in range(ND):
                nc.sync.dma_start(
                    out=xT[:, di, :], in_=xt16[di * 128:(di + 1) * 128, :]
                )

            vT = big.tile([128, NT, H, 129], BF16, tag="vT")
            nc.vector.memset(vT[:, :, :, 128:129], 1.0)
            oT = big.tile([128, H, S], BF16, tag="oT")

            # ================= emit helpers =================

            def emit_v(l, st_lo, st_hi):
                wv = vtiles[l]
                bvt = biases[l]["bv"]
                dqv = biases[l]["dqv"]
                for st in range(st_lo, st_hi):
                    for dc in range(NSC):
                        ps = p_mm.tile([128, 4, 128], F32, tag="mm")
                        for kt in range(KT2):
                            nc.tensor.matmul(
                                ps,
                                lhsT=x8[:, 2 * kt:2 * kt + 2,
                                        st * 128:(st + 1) * 128],
                                rhs=wv[dc][:, kt],
                                start=(kt == 0), stop=(kt == KT2 - 1),
                                perf_mode=DR,
                            )
                        with nc.allow_low_precision(reason="bf16 v"):
                            nc.vector.scalar_tensor_tensor(
                                out=vT[:, st, 4 * dc:4 * dc + 4, 0:128],
                                in0=ps, scalar=dqv,
                                in1=bvt[:, 4 * dc:4 * dc + 4, :],
                                op0=OP.mult, op1=OP.add,
                            )

            qks = {}

            def emit_qk(l, h, cs):
                """QK projections for head h, chunks cs (subset of {0,1})."""
                if (l, h) not in qks:
                    wq = wsm.tile([128, KT2, 2, 128], FP8, tag="wq",
                                  name=f"wq{l}_{h}", bufs=3)
                    nc.sync.dma_start(out=wq, in_=wqk8[l, 0, h])
                    wk = wsm.tile([128, KT2, 2, 128], FP8, tag="wk",
                                  name=f"wk{l}_{h}", bufs=3)
                    nc.sync.dma_start(out=wk, in_=wqk8[l, 1, h])
                    qb = qkp.tile([128, S], BF16, tag="qh", name=f"qb{l}_{h}")
                    kp = qkp.tile([128, S], BF16, tag="kh", name=f"kp{l}_{h}")
                    qks[(l, h)] = (wq, wk, qb, kp)
                wq, wk, qb, kp = qks[(l, h)]
                bq, bk = biases[l]["bq"], biases[l]["bk"]
                dqq, dqk = biases[l]["dqq"], biases[l]["dqk"]
                for c in cs:
                    sl = slice(c * SC, (c + 1) * SC)
                    psq = p_mm.tile([128, SC], F32, tag="mm")
                    for kt in range(KT2):
                        nc.tensor.matmul(
                            psq, lhsT=wq[:, kt],
                            rhs=x8[:, 2 * kt:2 * kt + 2, sl],
                            start=(kt == 0), stop=(kt == KT2 - 1),
                            perf_mode=DR,
                        )
                    nc.scalar.activation(
                        out=qb[:, sl], in_=psq, func=AF.Identity,
                        bias=bq[:, h:h + 1], scale=dqq,
                    )
                    psk = p_mm.tile([128, SC], F32, tag="mm")
                    for kt in range(KT2):
                        nc.tensor.matmul(
                            psk, lhsT=wk[:, kt],
                            rhs=x8[:, 2 * kt:2 * kt + 2, sl],
                            start=(kt == 0), stop=(kt == KT2 - 1),
                            perf_mode=DR,
                        )
                    nc.scalar.activation(
                        out=kp[:, sl], in_=psk, func=AF.Identity,
                        bias=bk[:, h:h + 1], scale=dqk,
                    )

            sms = {}

            def emit_scores(l, h, p_sc):
                """Transposed banded scores + exp for all key tiles of head h:
                sc[j', q'] for keys in tile jt, queries jt*128 .. +158."""
                _, _, qb, kp = qks[(l, h)]
                tiles = []
                for jt in range(NT):
                    nq = NQB if jt < NT - 1 else 128
                    scp = p_sc.tile([128, NQB], F32, tag="sc")
                    nc.tensor.matmul(
                        scp[:, 0:nq],
                        lhsT=kp[:, jt * 128:(jt + 1) * 128],
                        rhs=qb[:, jt * 128:jt * 128 + nq],
                        start=True, stop=True,
                    )
                    ex = smp.tile([128, 160], BF16, tag="ex")
                    nc.scalar.activation(
                        out=ex[:, 0:nq], in_=scp[:, 0:nq], func=AF.Exp,
                    )
                    with nc.allow_low_precision(reason="bf16 probs"):
                        nc.vector.tensor_mul(
                            ex[:, 0:nq], ex[:, 0:nq], bandt[:, 0:nq]
                        )
                    tiles.append(ex)
                sms[(l, h)] = tiles

            def emit_av(l, h, p_av, p_tr, mid=None):
                """AV (+denominator via ones column) for all query tiles of
                head h; av comes out [queries, hd+1]: normalize per-partition
                into ot tiles while the PE streams the next its, then
                transpose all tiles back into head-major oT. Transposes share
                the av pool slots (by the time they run, those avs are
                consumed)."""
                tiles = sms.pop((l, h))
                ots = []
                avs = {}

                def av_diag(it):
                    av = p_av.tile([128, 129], F32, tag="av")
                    avs[it] = av
                    nc.tensor.matmul(
                        av, lhsT=tiles[it][:, 0:128],
                        rhs=vT[:, it, h, :],
                        start=True, stop=(it == 0), skip_group_check=True,
                    )

                def av_prev(it):
                    # rows 0..95 of the prev tile's tail cols are band-masked
                    # zeros, so the full partition range contributes only the
                    # window overlap.
                    nc.tensor.matmul(
                        avs[it][0:31, :],
                        lhsT=tiles[it - 1][:, 128:159],
                        rhs=vT[:, it - 1, h, :],
                        start=False, stop=True, skip_group_check=True,
                    )

                def av_norm(it):
                    av = avs.pop(it)
                    rc = smp.tile([128, 1], F32, tag="rc")
                    nc.vector.reciprocal(rc, av[:, 128:129])
                    ot = smp.tile([128, 128], BF16, tag="ot")
                    with nc.allow_low_precision(reason="bf16 o"):
                        nc.vector.tensor_scalar(
                            out=ot, in0=av[:, 0:128],
                            scalar1=rc, scalar2=None, op0=OP.mult,
                        )
                    ots.append(ot)

                # stagger diag/prev so consecutive matmuls never target the
                # same PSUM region back-to-back (write-drain ~173ns).
                av_diag(0)
                av_norm(0)
                av_diag(1)
                for it in range(2, NT):
                    av_diag(it)
                    av_prev(it - 1)
                    av_norm(it - 1)
                av_prev(NT - 1)
                av_norm(NT - 1)
                if mid is not None:
                    mid()
                for it in range(NT):
                    trp = p_tr.tile([128, 128], BF16, tag="tr")
                    nc.tensor.transpose(trp, ots[it], idt)
                    nc.vector.tensor_copy(
                        out=oT[:, h, it * 128:(it + 1) * 128], in_=trp
                    )

            def load_wo(l, t):
                wo = wsm.tile([128, ND, 128], BF16, tag="wo", name=f"wo{l}_{t}",
                              bufs=4)
                nc.sync.dma_start(out=wo, in_=wot[l, t])
                return wo

            op_state = {}

            def emit_op_partial(l, c, t, di_hi, pool=None, tag="mm"):
                wo = load_wo(l, t)
                sl = slice(c * SC, (c + 1) * SC)
                ps = (pool or p_mm).tile([128, SC], F32, tag=tag)
                for di in range(di_hi):
                    nc.tensor.matmul(
                        ps, lhsT=wo[:, di, :], rhs=oT[:, di, sl],
                        start=(di == 0), stop=False,
                    )
                op_state[(c, t)] = (wo, ps)

            def emit_op_finish(l, c, t, di_lo):
                wo, ps = op_state.pop((c, t))
                sl = slice(c * SC, (c + 1) * SC)
                bo = biases[l]["bo"]
                for di in range(di_lo, ND):
                    nc.tensor.matmul(
                        ps, lhsT=wo[:, di, :], rhs=oT[:, di, sl],
                        start=(di == 0), stop=(di == ND - 1),
                    )
                with nc.allow_low_precision(reason="bf16 residual"):
                    nc.vector.scalar_tensor_tensor(
                        out=xT[:, t, sl], in0=ps,
                        scalar=bo[:, t:t + 1], in1=xT[:, t, sl],
                        op0=OP.add, op1=OP.add,
                    )

            def emit_op(l, c, t):
                emit_op_partial(l, c, t, 0)
                emit_op_finish(l, c, t, 0)

            # LN pieces
            def ln_alloc(nm):
                st1 = ln_ps.tile([1, SC], F32, tag="st1", name=f"st1{nm}")
                st2 = ln_ps.tile([1, SC], F32, tag="st2", name=f"st2{nm}")
                return st1, st2

            def ln_stats_di(st1, st2, c, di):
                sl = slice(c * SC, (c + 1) * SC)
                sq = roll.tile([128, SC], BF16, tag="sq")
                nc.scalar.activation(out=sq, in_=xT[:, di, sl], func=AF.Square)
                nc.tensor.matmul(
                    st1, lhsT=ones_col, rhs=xT[:, di, sl],
                    start=(di == 0), stop=(di == ND - 1),
                )
                nc.tensor.matmul(
                    st2, lhsT=ones_col, rhs=sq,
                    start=(di == 0), stop=(di == ND - 1),
                )

            def ln_stats(st1, st2, c):
                for di in range(ND):
                    ln_stats_di(st1, st2, c, di)

            def ln_rows_bcast(st1, st2, abt, bbt, c):
                """Broadcast the raw per-token sums to all partitions first
                (PE), then do mean/var/sqrt/reciprocal on [128, SC] tiles —
                single-partition DVE math (esp. reciprocal) is ~6x slower."""
                r1 = lnp.tile([1, SC], BF16, tag="r1")
                r2 = lnp.tile([1, SC], BF16, tag="r2")
                nc.scalar.activation(out=r1, in_=st1, func=AF.Identity)
                nc.scalar.activation(out=r2, in_=st2, func=AF.Identity)
                bc1 = ln_bc.tile([128, SC], F32, tag="bc")
                nc.tensor.matmul(bc1, lhsT=ones_row, rhs=r1,
                                 start=True, stop=True)
                bc2 = ln_bc.tile([128, SC], F32, tag="bc")
                nc.tensor.matmul(bc2, lhsT=ones_row, rhs=r2,
                                 start=True, stop=True)
                mw = lnp.tile([128, SC], F32, tag="mw")
                nc.vector.tensor_scalar(
                    out=mw, in0=bc1, scalar1=1.0 / D, scalar2=None, op0=OP.mult,
                )
                m2w = lnp.tile([128, SC], F32, tag="m2w")
                nc.vector.tensor_mul(m2w, mw, mw)
                vw = lnp.tile([128, SC], F32, tag="vw")
                nc.vector.scalar_tensor_tensor(
                    out=vw, in0=bc2, scalar=1.0 / D, in1=m2w,
                    op0=OP.mult, op1=OP.subtract,
                )
                nc.scalar.activation(
                    out=vw, in_=vw, func=AF.Sqrt, bias=epst[:, 0:1],
                )
                with nc.allow_low_precision(reason="bf16 rstd within tolerance"):
                    nc.vector.reciprocal(abt[:, c, :], vw)
                with nc.allow_low_precision(reason="bf16 LN shift"):
                    nc.vector.scalar_tensor_tensor(
                        out=bbt[:, c, :], in0=mw, scalar=-1.0,
                        in1=abt[:, c, :], op0=OP.mult, op1=OP.mult,
                    )

            def ln_apply(abt, bbt, gt, ht, c, x8out=False):
                sl = slice(c * SC, (c + 1) * SC)
                for di in range(ND):
                    d = xT[:, di, sl]
                    nc.vector.tensor_mul(d, d, abt[:, c, :])
                    nc.vector.tensor_add(d, d, bbt[:, c, :])
                    if not skip_affine:
                        nc.vector.tensor_scalar(
                            out=d, in0=d,
                            scalar1=gt[:, di:di + 1], scalar2=ht[:, di:di + 1],
                            op0=OP.mult, op1=OP.add,
                        )
                    if x8out:
                        with nc.allow_low_precision(reason="fp8 gemm operand"):
                            nc.scalar.activation(
                                out=x8[:, di, sl], in_=d, func=AF.Identity,
                            )

            def emit_w1(l, c, ft_lo, ft_hi):
                sl = slice(c * SC, (c + 1) * SC)
                b1 = biases[l]["b1"]
                for ft in range(ft_lo, ft_hi):
                    w1 = w1p.tile([128, ND, 128], BF16, tag="w1",
                                  name=f"w1_{l}_{c}_{ft}")
                    nc.sync.dma_start(out=w1, in_=w1t[l, ft])
                    ps = p_mm.tile([128, SC], F32, tag="mm")
                    for di in range(ND):
                        nc.tensor.matmul(
                            ps, lhsT=w1[:, di, :], rhs=xT[:, di, sl],
                            start=(di == 0), stop=(di == ND - 1),
                        )
                    nc.scalar.activation(
                        out=hT[0][:, ft, :], in_=ps, func=AF.Relu,
                        bias=b1[:, ft:ft + 1],
                    )

            hT = [None]

            def emit_w2(l, c, st12=None):
                sl = slice(c * SC, (c + 1) * SC)
                b2 = biases[l]["b2"]
                for t in range(ND):
                    if st12 is not None and t >= 1:
                        ln_stats_di(st12[0], st12[1], c, t - 1)
                    w2 = wpool.tile([128, NF, 128], BF16, tag="w2",
                                    name=f"w2_{l}_{c}_{t}", bufs=3)
                    nc.sync.dma_start(out=w2, in_=w2t[l, t])
                    ps = p_mm.tile([128, SC], F32, tag="mm")
                    for ft in range(NF):
                        nc.tensor.matmul(
                            ps, lhsT=w2[:, ft, :], rhs=hT[0][:, ft, :],
                            start=(ft == 0), stop=(ft == NF - 1),
                        )
                    with nc.allow_low_precision(reason="bf16 residual"):
                        nc.vector.scalar_tensor_tensor(
                            out=xT[:, t, sl], in0=ps,
                            scalar=b2[:, t:t + 1], in1=xT[:, t, sl],
                            op0=OP.add, op1=OP.add,
                        )
                if st12 is not None:
                    ln_stats_di(st12[0], st12[1], c, ND - 1)

            wh = cst.tile([128, ND, NOUT], BF16, tag="wh")
            nc.sync.dma_start(out=wh, in_=wht[:, :, :])

            def emit_head(st_lo, st_hi):
                for st in range(st_lo, st_hi):
                    ps = p_mm.tile([128, NOUT], F32, tag="mm")
                    for di in range(ND):
                        nc.tensor.matmul(
                            ps, lhsT=xT[:, di, st * 128:(st + 1) * 128],
                            rhs=wh[:, di, :],
                            start=(di == 0), stop=(di == ND - 1),
                        )
                    ob = roll.tile([128, NOUT], F32, tag="ob")
                    nc.vector.tensor_add(ob, ps, bhbt)
                    nc.sync.dma_start(out=y[st * 128:(st + 1) * 128, :], in_=ob)

            def tap(name, ap):
                if taps is None or name not in taps:
                    return
                t = nc.dram_tensor(f"tap_{name}", list(ap.shape), ap.dtype,
                                   kind="ExternalOutput")
                sl = tuple(slice(0, d) for d in ap.shape)
                nc.sync.dma_start(out=t[sl], in_=ap)

            # ================= main flow =================

            # layer-0 entry (no boundary cover needed)
            emit_v(0, 0, NT)
            emit_qk(0, 0, (0, 1))
            emit_qk(0, 1, (0, 1))
            emit_qk(0, 2, (0, 1))
            tap("vT", vT)
            tap("qb00", qks[(0, 0)][2])
            tap("kp00", qks[(0, 0)][3])

            for l in range(L):
                # ---- attention phase
                with tc.tile_pool(name="p_sc", bufs=2, space="PSUM") as p_sc, \
                     tc.tile_pool(name="p_av", bufs=2, space="PSUM") as p_av, \
                     tc.tile_pool(name="p_tr", bufs=1, space="PSUM") as p_tr:
                    emit_scores(l, 0, p_sc)
                    if l == 0:
                        tap("ex0", sms[(0, 0)][0])
                        tap("ex3", sms[(0, 0)][3])
                    for h in range(1, H):
                        if h + 2 < H:
                            emit_qk(l, h + 2, (0, 1))
                        emit_scores(l, h, p_sc)
                        if h == H - 2:
                            emit_op_partial(l, 0, 0, 5)
                        elif h == H - 1:
                            emit_op_partial(l, 0, 1, 6)
                        emit_av(l, h - 1, p_av, p_tr)
                    emit_op_partial(l, 0, 2, 7)
                    emit_av(l, H - 1, p_av, p_tr)
                    # filler while the last head's normalize/copy chain
                    # drains on vector: heads 0..4 of the remaining out-proj
                    # tiles are ready; park them in the draining attention
                    # PSUM pools (p_mm's 3 bufs are all held).
                    emit_op_partial(l, 0, 3, 5, pool=p_sc, tag="sc")
                    emit_op_partial(l, 0, 4, 5, pool=p_sc, tag="sc")
                    emit_op_partial(l, 0, 5, 5, pool=p_av, tag="av")
                    emit_op_partial(l, 0, 6, 5, pool=p_av, tag="av")
                    emit_op_partial(l, 0, 7, 5, pool=p_tr, tag="tr")
                    emit_op_finish(l, 0, 0, 5)
                    emit_op_finish(l, 0, 1, 6)
                    emit_op_finish(l, 0, 2, 7)
                    for t in range(3, ND):
                        emit_op_finish(l, 0, t, 5)

                if l == 0:
                    tap("oT0", oT)
                    tap("xT0", xT)
                # ---- LN1 + FFN + LN2 (+ next-layer V/QK or head as cover)
                with tc.tile_pool(name="ln_ps", bufs=1, space="PSUM") as ln_ps, \
                     tc.tile_pool(name="ln_bc", bufs=2, space="PSUM") as ln_bc:
                    gb = biases[l]
                    abt = lnp.tile([128, NSC, SC], BF16, tag="abt")
                    bbt = lnp.tile([128, NSC, SC], BF16, tag="bbt")

                    st10, st20 = ln_alloc(f"a{l}0")
                    for t in range(4):
                        emit_op(l, 1, t)
                        ln_stats_di(st10, st20, 0, 2 * t)
                        ln_stats_di(st10, st20, 0, 2 * t + 1)
                    emit_op(l, 1, 4)
                    ln_rows_bcast(st10, st20, abt, bbt, 0)
                    emit_op(l, 1, 5)
                    emit_op(l, 1, 6)
                    emit_op(l, 1, 7)
                    ln_apply(abt, bbt, gb["g1"], gb["h1"], 0)
                    st11, st21 = ln_alloc(f"a{l}1")
                    ln_stats(st11, st21, 1)
                    hT[0] = hpool.tile([128, NF, SC], BF16, tag="hT",
                                       name=f"hT{l}_0")
                    emit_w1(l, 0, 0, 4)
                    ln_rows_bcast(st11, st21, abt, bbt, 1)
                    ln_apply(abt, bbt, gb["g1"], gb["h1"], 1)
                    emit_w1(l, 0, 4, NF)
                    if l + 1 < L:
                        load_biases(l + 1)
                    st1b, st2b = ln_alloc(f"b{l}0")
                    emit_w2(l, 0, st12=(st1b, st2b))
                    hT[0] = hpool.tile([128, NF, SC], BF16, tag="hT",
                                       name=f"hT{l}_1")
                    emit_w1(l, 1, 0, 16)
                    ln_rows_bcast(st1b, st2b, abt, bbt, 0)
                    emit_w1(l, 1, 16, NF)
                    if l + 1 < L:
                        load_wv(l + 1)
                    st1c, st2c = ln_alloc(f"b{l}1")
                    emit_w2(l, 1, st12=(st1c, st2c))
                    ln_apply(abt, bbt, gb["g2"], gb["h2"], 0,
                             x8out=(l + 1 < L))
                    if l + 1 < L:
                        emit_v(l + 1, 0, 2)
                        emit_qk(l + 1, 0, (0,))
                        ln_rows_bcast(st1c, st2c, abt, bbt, 1)
                        emit_v(l + 1, 2, 4)
                        emit_qk(l + 1, 1, (0,))
                        emit_qk(l + 1, 2, (0,))
                        ln_apply(abt, bbt, gb["g2"], gb["h2"], 1, x8out=True)
                        emit_v(l + 1, 4, NT)
                        emit_qk(l + 1, 0, (1,))
                        emit_qk(l + 1, 1, (1,))
                        emit_qk(l + 1, 2, (1,))
                    else:
                        emit_head(0, 2)
                        ln_rows_bcast(st1c, st2c, abt, bbt, 1)
                        emit_head(2, 4)
                        ln_apply(abt, bbt, gb["g2"], gb["h2"], 1)
                        emit_head(4, NT)

    _split_multi_waits(nc)
    return nc


def _host_prep(inputs):
    """Pre-transpose / quantize weights, fold pos into X. Layout work."""
    import ml_dtypes
    bf = ml_dtypes.bfloat16
    e4 = ml_dtypes.float8_e4m3
    f32 = np.float32
    inp = {k: np.asarray(v, dtype=f32) for k, v in inputs.items()}

    pos = np.arange(S, dtype=f32)[:, None]
    div = np.exp(np.arange(0, D, 2, dtype=f32) * (-np.log(10000.0) / D)).astype(f32)
    pe = np.zeros((S, D), f32)
    pe[:, 0::2] = np.sin(pos * div)
    pe[:, 1::2] = np.cos(pos * div)

    # band mask: key tile partitions j'=0..127, query cols q'=0..158 (+pad)
    jj = np.arange(128)[:, None]
    qq = np.arange(160)[None, :]
    band01 = (((qq - jj) >= 0) & ((qq - jj) < WIN)).astype(f32)

    def colmajor(v):  # [L, X] -> [L, 128, X/128] col slices
        return np.ascontiguousarray(v.reshape(L, -1, 128).transpose(0, 2, 1))

    def lhsT_tiles(wT, n_out):
        # wT: [L, K, M] (w^T); -> [L, n_out, 128, K/128, 128] where
        # [l, t, p, ki, m] = wT[l, 128*ki + p, 128*t + m]
        Lw, Kw, Mw = wT.shape
        assert Mw == n_out * 128
        r = wT.reshape(Lw, Kw // 128, 128, n_out, 128)
        return np.ascontiguousarray(r.transpose(0, 3, 2, 1, 4))

    def wscale(w):
        rms = np.sqrt(np.mean(w.astype(np.float64) ** 2))
        return float(2.0 ** np.round(np.log2(2.0 / max(rms, 1e-30))))

    # fp8 DoubleRow tiles: pairs of K-tiles (2kt, 2kt+1) on free dim 1
    def qk_tiles8(wT, s):  # wT [D, D] -> [H, 128, KT2, 2, 128]
        r = (wT * s).reshape(KT2, 2, 128, H, 128)
        return np.ascontiguousarray(r.transpose(3, 2, 0, 1, 4)).astype(e4)

    def v_tiles8(wT, s):  # wT [D, D] -> [NSC, 128, KT2, 2, SC]
        r = (wT * s).reshape(KT2, 2, 128, NSC, SC)
        return np.ascontiguousarray(r.transpose(3, 2, 0, 1, 4)).astype(e4)

    wqk8 = np.empty((L, 2, H, 128, KT2, 2, 128), e4)
    wv8 = np.empty((L, NSC, 128, KT2, 2, SC), e4)
    dqqv = np.empty((L, 128, 3), f32)
    for l in range(L):
        sq = wscale(inp["Wq"][l])
        sk = wscale(inp["Wk"][l])
        sv = wscale(inp["Wv"][l])
        wqk8[l, 0] = qk_tiles8(inp["Wq"][l].T, sq)
        wqk8[l, 1] = qk_tiles8(inp["Wk"][l].T, sk)
        wv8[l] = v_tiles8(inp["Wv"][l].T, sv)
        dqqv[l, :, 0] = SCALE / sq
        dqqv[l, :, 1] = 1.0 / sk
        dqqv[l, :, 2] = 1.0 / sv

    skip_affine = bool(
        np.all(inp["ln1_g"] == 1.0) and np.all(inp["ln1_b"] == 0.0)
        and np.all(inp["ln2_g"] == 1.0) and np.all(inp["ln2_b"] == 0.0)
    )

    shared = {
        "band": band01.astype(bf),
        "ident": np.eye(128, dtype=f32).astype(bf),
        "wqk8": wqk8,
        "wv8": wv8,
        "dqqv": dqqv,
        "wot": lhsT_tiles(inp["Wo"].transpose(0, 2, 1), ND).astype(bf),
        "w1t": lhsT_tiles(inp["W1"].transpose(0, 2, 1), NF).astype(bf),
        "w2t": lhsT_tiles(inp["W2"].transpose(0, 2, 1), ND).astype(bf),
        "wht": np.ascontiguousarray(
            inp["Wh"].T.reshape(ND, 128, NOUT).transpose(1, 0, 2)).astype(bf),
        "bqs": colmajor(inp["bq"] * SCALE),
        "bkc": colmajor(inp["bk"]),
        "bvb": inp["bv"].reshape(L, 1, D).astype(bf),
        "boc": colmajor(inp["bo"]),
        "b1c": colmajor(inp["b1"]),
        "b2c": colmajor(inp["b2"]),
        "g1c": colmajor(inp["ln1_g"]),
        "h1c": colmajor(inp["ln1_b"]),
        "g2c": colmajor(inp["ln2_g"]),
        "h2c": colmajor(inp["ln2_b"]),
        "onesc": np.ones((128, 1), f32).astype(bf),
        "onesr": np.ones((1, 128), f32).astype(bf),
        "bhb": np.ascontiguousarray(inp["bh"].reshape(1, NOUT)),
    }
    in_maps = []
    for b in range(N_CORES):
        xb = (inp["X"][b] + pe).T  # [D, S]
        m = dict(shared)
        m["xt16"] = np.ascontiguousarray(xb).astype(bf)
        m["xt8"] = np.ascontiguousarray(
            xb.reshape(ND, 128, S).transpose(1, 0, 2)).astype(e4)
        in_maps.append(m)
    return in_maps, skip_affine


_NC_CACHE = {}


def run(inputs, trace=False, **spmd_kwargs):
    in_maps, skip_affine = _host_prep(inputs)
    key = ("nc", skip_affine)
    if key not in _NC_CACHE:
        _NC_CACHE[key] = build_program(skip_affine=skip_affine)
    nc = _NC_CACHE[key]
    res = run_bass_kernel_spmd(
        nc, in_maps, list(range(N_CORES)), trace=trace, **spmd_kwargs
    )
    out = np.concatenate([res.results[i]["out"] for i in range(N_CORES)], axis=0)
    return out, res


def kernel(**inputs) -> np.ndarray:
    out, _ = run(inputs, trace=False)
    return out


# revision 6
# speedup vs baseline: 1.0102x; 1.0102x over previous
"""LocalWindowTransformer Trainium2 kernel, v3.

Data-parallel over batch (B=8 -> 8 cores). bf16 datapath with fp8
(e4m3 DoubleRow) QKV projections: K=1024 contractions run as 4
pair-packed K=256 matmuls at 2x PE throughput (weights pre-scaled to
e4m3 range on host, dequant folded into the PSUM-drain activations).
Attention is computed in transposed-score form: scores land
[keys, queries] (one matmul per 128-key tile over its 159-query band,
window=32), exp'd and band-masked, and the AV matmul consumes them
directly; the AV rhs carries a ones column so the softmax denominator
falls out of the same matmul, and a per-tile reciprocal+scale
normalizes before one transpose back into head-major oT. LN row stats
accumulate via ones-column matmuls interleaved with the GEMM streams;
the row math (mean/var/rsqrt) runs on broadcast [128,SC] tiles because
single-partition DVE ops are ~6x slower. Weight pools are deep enough
(w1p=6, w2/wo multi-buf) to keep the PE from draining its DVFS ramp.
PSUM accumulation and LN row stats stay f32.
"""

import numpy as np

import concourse.bass as bass
import concourse.tile as tile
from concourse import mybir
from concourse.bass_utils import run_bass_kernel_spmd
from concourse.vector_clock import ScopedClock

F32 = mybir.dt.float32
BF16 = mybir.dt.bfloat16
FP8 = mybir.dt.float8e4
DR = mybir.MatmulPerfMode.DoubleRow
AF = mybir.ActivationFunctionType
OP = mybir.AluOpType

B, S, D = 8, 1024, 1024
H, HD, L, FF, NOUT, WIN = 8, 128, 4, 4096, 512, 32
ND = D // 128     # 8 feature tiles
NT = S // 128     # 8 token tiles
NF = FF // 128    # 32 ffn tiles
KT2 = ND // 2     # 4 feature-pair tiles (fp8 DoubleRow)
SC = 512          # s-chunk
NSC = S // SC     # 2
NQB = 159         # banded query cols per key tile (128 + WIN - 1)
SCALE = 1.0 / float(np.sqrt(HD))
EPS = 1e-5
N_CORES = 8


class SafeTileContext(tile.TileContext):
    """walrus in this image rejects a Drain carrying several sem waits
    ("Too many sync wait commands"). Absorb the outstanding waits into
    single-wait SP nops before the tail drain so the drain itself is
    wait-free."""

    def _drain_and_barrier(self, tick_clock, wait_clock):
        gclock = tick_clock.global_clock
        for proc in range(len(gclock)):
            tick = gclock[proc]
            if tick > 0:
                partial = ScopedClock()
                partial.require_at_least(None, proc, tick)
                nop = self.nc.sync.nop(nofuse=True)
                wait_clock.add_sem_waits(nop.ins, partial)
        self.nc.sync.drain()
        self.nc.all_engine_barrier()
        popped = self.nc._tile_sem_poison_stack.pop()
        assert popped is self._sem_poison
        self.nc.clear_and_free_semaphores(list(self.sems.allocated().values()))
        self.nc.all_engine_barrier()


def _split_multi_waits(nc):
    """This image's walrus accepts at most ONE sem wait per instruction.
    Hoist extra waits onto same-engine NoOps immediately preceding the
    instruction."""
    n = 0
    for f in nc.m.functions:
        for bb in f.blocks:
            insts = bb.instructions
            out = []
            for inst in insts:
                si = inst.sync_info
                waits = list(si.on_wait) if si is not None else []
                if len(waits) > 1:
                    for w in waits[:-1]:
                        n += 1
                        nop = mybir.InstNoOp(
                            name=f"{inst.name}-wsplit{n}",
                            engine=inst.engine,
                            ins=[], outs=[],
                            sync_info=mybir.SyncInfo(on_wait=[w], on_update=[]),
                        )
                        out.append(nop)
                    inst.sync_info = mybir.SyncInfo(
                        on_wait=[waits[-1]], on_update=list(si.on_update)
                    )
                out.append(inst)
            insts[:] = out
    return n


def build_program(skip_affine=False, taps=None):
    nc = bass.Bass()

    xt16 = nc.dram_tensor("xt16", [D, S], BF16, kind="ExternalInput")
    xt8 = nc.dram_tensor("xt8", [128, ND, S], FP8, kind="ExternalInput")
    band = nc.dram_tensor("band", [128, 160], BF16, kind="ExternalInput")
    ident = nc.dram_tensor("ident", [128, 128], BF16, kind="ExternalInput")
    wqk8 = nc.dram_tensor("wqk8", [L, 2, H, 128, KT2, 2, 128], FP8,
                          kind="ExternalInput")
    wv8 = nc.dram_tensor("wv8", [L, NSC, 128, KT2, 2, SC], FP8,
                         kind="ExternalInput")
    wot = nc.dram_tensor("wot", [L, ND, 128, ND, 128], BF16, kind="ExternalInput")
    w1t = nc.dram_tensor("w1t", [L, NF, 128, ND, 128], BF16, kind="ExternalInput")
    w2t = nc.dram_tensor("w2t", [L, ND, 128, NF, 128], BF16, kind="ExternalInput")
    wht = nc.dram_tensor("wht", [128, ND, NOUT], BF16, kind="ExternalInput")
    bqs = nc.dram_tensor("bqs", [L, 128, H], F32, kind="ExternalInput")  # bq*SCALE
    bkc = nc.dram_tensor("bkc", [L, 128, H], F32, kind="ExternalInput")
    bvb = nc.dram_tensor("bvb", [L, 1, D], BF16, kind="ExternalInput")
    dqqv = nc.dram_tensor("dqqv", [L, 128, 3], F32, kind="ExternalInput")
    boc = nc.dram_tensor("boc", [L, 128, ND], F32, kind="ExternalInput")
    b1c = nc.dram_tensor("b1c", [L, 128, NF], F32, kind="ExternalInput")
    b2c = nc.dram_tensor("b2c", [L, 128, ND], F32, kind="ExternalInput")
    g1c = nc.dram_tensor("g1c", [L, 128, ND], F32, kind="ExternalInput")
    h1c = nc.dram_tensor("h1c", [L, 128, ND], F32, kind="ExternalInput")
    g2c = nc.dram_tensor("g2c", [L, 128, ND], F32, kind="ExternalInput")
    h2c = nc.dram_tensor("h2c", [L, 128, ND], F32, kind="ExternalInput")
    onesc = nc.dram_tensor("onesc", [128, 1], BF16, kind="ExternalInput")
    onesr = nc.dram_tensor("onesr", [1, 128], BF16, kind="ExternalInput")
    bhb = nc.dram_tensor("bhb", [1, NOUT], F32, kind="ExternalInput")
    y = nc.dram_tensor("out", [S, NOUT], F32, kind="ExternalOutput")

    with SafeTileContext(nc) as tc:
        from contextlib import ExitStack

        with ExitStack() as ctx:
            ep = ctx.enter_context
            p_mm = ep(tc.tile_pool(name="p_mm", bufs=3, space="PSUM"))
            wpool = ep(tc.tile_pool(name="wpool", bufs=2))
            wsm = ep(tc.tile_pool(name="wsm", bufs=2))
            w1p = ep(tc.tile_pool(name="w1p", bufs=6))
            big = ep(tc.tile_pool(name="big", bufs=1))
            hpool = ep(tc.tile_pool(name="hpool", bufs=1))
            qkp = ep(tc.tile_pool(name="qkp", bufs=3))
            smp = ep(tc.tile_pool(name="smp", bufs=20))
            cst = ep(tc.tile_pool(name="cst", bufs=1))
            bias = ep(tc.tile_pool(name="bias", bufs=2))
            lnp = ep(tc.tile_pool(name="lnp", bufs=1))
            roll = ep(tc.tile_pool(name="roll", bufs=2))

            # ---- input + layer-0 weights first so compute starts ASAP
            x8 = big.tile([128, ND, S], FP8, tag="x8")
            for kt in range(KT2):
                nc.sync.dma_start(
                    out=x8[:, 2 * kt:2 * kt + 2, :],
                    in_=xt8[:, 2 * kt:2 * kt + 2, :],
                )

            biases = {}

            def load_biases(l):
                t = {}
                for nm, src, shape, dt in (
                    ("bq", bqs, [128, H], F32), ("bk", bkc, [128, H], F32),
                    ("bo", boc, [128, ND], F32), ("b1", b1c, [128, NF], F32),
                    ("b2", b2c, [128, ND], F32), ("g1", g1c, [128, ND], F32),
                    ("h1", h1c, [128, ND], F32), ("g2", g2c, [128, ND], F32),
                    ("h2", h2c, [128, ND], F32),
                ):
                    tl = bias.tile(shape, dt, tag=nm, name=f"{nm}_{l}")
                    nc.sync.dma_start(out=tl, in_=src[l])
                    t[nm] = tl
                dq = bias.tile([128, 3], F32, tag="dq", name=f"dq_{l}")
                nc.sync.dma_start(out=dq, in_=dqqv[l])
                t["dqq"], t["dqk"], t["dqv"] = dq[:, 0:1], dq[:, 1:2], dq[:, 2:3]
                bvt = bias.tile([128, ND, 128], BF16, tag="bv", name=f"bv_{l}")
                nc.sync.dma_start(
                    out=bvt, in_=bvb[l].to_broadcast([128, D])
                )
                t["bv"] = bvt
                biases[l] = t

            load_biases(0)

            vtiles = {}

            def load_wv(l):
                wv = []
                for dc in range(NSC):
                    wt = wpool.tile([128, KT2, 2, SC], FP8, tag="wv",
                                    name=f"wv{l}_{dc}")
                    nc.sync.dma_start(out=wt, in_=wv8[l, dc])
                    wv.append(wt)
                vtiles[l] = wv

            load_wv(0)

            # ---- constants (after the hot-path DMAs)
            bandt = cst.tile([128, 160], BF16, tag="bandt")
            nc.sync.dma_start(out=bandt, in_=band[:, :])
            idt = cst.tile([128, 128], BF16, tag="idt")
            nc.sync.dma_start(out=idt, in_=ident[:, :])
            ones_col = cst.tile([128, 1], BF16, tag="onc")
            nc.sync.dma_start(out=ones_col, in_=onesc[:, :])
            ones_row = cst.tile([1, 128], BF16, tag="onr")
            nc.sync.dma_start(out=ones_row, in_=onesr[:, :])
            bhbt = cst.tile([128, NOUT], F32, tag="bhb")
            nc.sync.dma_start(out=bhbt, in_=bhb[0:1, :].to_broadcast([128, NOUT]))
            epst = cst.tile([128, 1], F32, tag="eps")
            nc.vector.memset(epst, EPS)

            xT = big.tile([128, ND, S], BF16, tag="xT")
            for di in range(ND):
                nc.sync.dma_start(
                    out=xT[:, di, :], in_=xt16[di * 128:(di + 1) * 128, :]
                )

            vT = big.tile([128, NT, H, 129], BF16, tag="vT")
            nc.vector.memset(vT[:, :, :, 128:129], 1.0)
            oT = big.tile([128, H, S], BF16, tag="oT")

            # ================= emit helpers =================

            def emit_v(l, st_lo, st_hi):
                wv = vtiles[l]
                bvt = biases[l]["bv"]
                dqv = biases[l]["dqv"]
                for st in range(st_lo, st_hi):
                    for dc in range(NSC):
                        ps = p_mm.tile([128, 4, 128], F32, tag="mm")
                        for kt in range(KT2):
                            nc.tensor.matmul(
                                ps,
                                lhsT=x8[:, 2 * kt:2 * kt + 2,
                                        st * 128:(st + 1) * 128],
                                rhs=wv[dc][:, kt],
                                start=(kt == 0), stop=(kt == KT2 - 1),
                                perf_mode=DR,
                            )
                        with nc.allow_low_precision(reason="bf16 v"):
                            nc.vector.scalar_tensor_tensor(
                                out=vT[:, st, 4 * dc:4 * dc + 4, 0:128],
                                in0=ps, scalar=dqv,
                                in1=bvt[:, 4 * dc:4 * dc + 4, :],
                                op0=OP.mult, op1=OP.add,
                            )

            qks = {}

            def emit_qk(l, h, cs):
                """QK projections for head h, chunks cs (subset of {0,1})."""
                if (l, h) not in qks:
                    wq = wsm.tile([128, KT2, 2, 128], FP8, tag="wq",
                                  name=f"wq{l}_{h}", bufs=3)
                    nc.sync.dma_start(out=wq, in_=wqk8[l, 0, h])
                    wk = wsm.tile([128, KT2, 2, 128], FP8, tag="wk",
                                  name=f"wk{l}_{h}", bufs=3)
                    nc.sync.dma_start(out=wk, in_=wqk8[l, 1, h])
                    qb = qkp.tile([128, S], BF16, tag="qh", name=f"qb{l}_{h}")
                    kp = qkp.tile([128, S], BF16, tag="kh", name=f"kp{l}_{h}")
                    qks[(l, h)] = (wq, wk, qb, kp)
                wq, wk, qb, kp = qks[(l, h)]
                bq, bk = biases[l]["bq"], biases[l]["bk"]
                dqq, dqk = biases[l]["dqq"], biases[l]["dqk"]
                for c in cs:
                    sl = slice(c * SC, (c + 1) * SC)
                    psq = p_mm.tile([128, SC], F32, tag="mm")
                    for kt in range(KT2):
                        nc.tensor.matmul(
                            psq, lhsT=wq[:, kt],
                            rhs=x8[:, 2 * kt:2 * kt + 2, sl],
                            start=(kt == 0), stop=(kt == KT2 - 1),
                            perf_mode=DR,
                        )
                    nc.scalar.activation(
                        out=qb[:, sl], in_=psq, func=AF.Identity,
                        bias=bq[:, h:h + 1], scale=dqq,
                    )
                    psk = p_mm.tile([128, SC], F32, tag="mm")
                    for kt in range(KT2):
                        nc.tensor.matmul(
                            psk, lhsT=wk[:, kt],
                            rhs=x8[:, 2 * kt:2 * kt + 2, sl],
                            start=(kt == 0), stop=(kt == KT2 - 1),
                            perf_mode=DR,
                        )
                    nc.scalar.activation(
                        out=kp[:, sl], in_=psk, func=AF.Identity,
                        bias=bk[:, h:h + 1], scale=dqk,
                    )

            sms = {}

            def emit_scores(l, h, p_sc):
                """Transposed banded scores + exp for all key tiles of head h:
                sc[j', q'] for keys in tile jt, queries jt*128 .. +158."""
                _, _, qb, kp = qks[(l, h)]
                tiles = []
                for jt in range(NT):
                    nq = NQB if jt < NT - 1 else 128
                    scp = p_sc.tile([128, NQB], F32, tag="sc")
                    nc.tensor.matmul(
                        scp[:, 0:nq],
                        lhsT=kp[:, jt * 128:(jt + 1) * 128],
                        rhs=qb[:, jt * 128:jt * 128 + nq],
                        start=True, stop=True,
                    )
                    ex = smp.tile([128, 160], BF16, tag="ex")
                    nc.scalar.activation(
                        out=ex[:, 0:nq], in_=scp[:, 0:nq], func=AF.Exp,
                    )
                    with nc.allow_low_precision(reason="bf16 probs"):
                        nc.vector.tensor_mul(
                            ex[:, 0:nq], ex[:, 0:nq], bandt[:, 0:nq]
                        )
                    tiles.append(ex)
                sms[(l, h)] = tiles

            def emit_av(l, h, p_av, p_tr, mid=None):
                """AV (+denominator via ones column) for all query tiles of
                head h; av comes out [queries, hd+1]: normalize per-partition
                into ot tiles while the PE streams the next its, then
                transpose all tiles back into head-major oT. Transposes share
                the av pool slots (by the time they run, those avs are
                consumed)."""
                tiles = sms.pop((l, h))
                ots = []
                avs = {}

                def av_diag(it):
                    av = p_av.tile([128, 129], F32, tag="av")
                    avs[it] = av
                    nc.tensor.matmul(
                        av, lhsT=tiles[it][:, 0:128],
                        rhs=vT[:, it, h, :],
                        start=True, stop=(it == 0), skip_group_check=True,
                    )

                def av_prev(it):
                    # rows 0..95 of the prev tile's tail cols are band-masked
                    # zeros, so the full partition range contributes only the
                    # window overlap.
                    nc.tensor.matmul(
                        avs[it][0:31, :],
                        lhsT=tiles[it - 1][:, 128:159],
                        rhs=vT[:, it - 1, h, :],
                        start=False, stop=True, skip_group_check=True,
                    )

                def av_norm(it):
                    av = avs.pop(it)
                    rc = smp.tile([128, 1], F32, tag="rc")
                    nc.vector.reciprocal(rc, av[:, 128:129])
                    ot = smp.tile([128, 128], BF16, tag="ot")
                    with nc.allow_low_precision(reason="bf16 o"):
                        nc.vector.tensor_scalar(
                            out=ot, in0=av[:, 0:128],
                            scalar1=rc, scalar2=None, op0=OP.mult,
                        )
                    ots.append(ot)

                # stagger diag/prev so consecutive matmuls never target the
                # same PSUM region back-to-back (write-drain ~173ns).
                av_diag(0)
                av_norm(0)
                av_diag(1)
                for it in range(2, NT):
                    av_diag(it)
                    av_prev(it - 1)
                    av_norm(it - 1)
                av_prev(NT - 1)
                av_norm(NT - 1)
                if mid is not None:
                    mid()
                for it in range(NT):
                    trp = p_tr.tile([128, 128], BF16, tag="tr")
                    nc.tensor.transpose(trp, ots[it], idt)
                    nc.vector.tensor_copy(
                        out=oT[:, h, it * 128:(it + 1) * 128], in_=trp
                    )

            def load_wo(l, t):
                wo = wsm.tile([128, ND, 128], BF16, tag="wo", name=f"wo{l}_{t}",
                              bufs=4)
                nc.sync.dma_start(out=wo, in_=wot[l, t])
                return wo

            op_state = {}

            def emit_op_partial(l, c, t, di_hi, pool=None, tag="mm"):
                wo = load_wo(l, t)
                sl = slice(c * SC, (c + 1) * SC)
                ps = (pool or p_mm).tile([128, SC], F32, tag=tag)
                for di in range(di_hi):
                    nc.tensor.matmul(
                        ps, lhsT=wo[:, di, :], rhs=oT[:, di, sl],
                        start=(di == 0), stop=False,
                    )
                op_state[(c, t)] = (wo, ps)

            def emit_op_finish(l, c, t, di_lo):
                wo, ps = op_state.pop((c, t))
                sl = slice(c * SC, (c + 1) * SC)
                bo = biases[l]["bo"]
                for di in range(di_lo, ND):
                    nc.tensor.matmul(
                        ps, lhsT=wo[:, di, :], rhs=oT[:, di, sl],
                        start=(di == 0), stop=(di == ND - 1),
                    )
                with nc.allow_low_precision(reason="bf16 residual"):
                    nc.vector.scalar_tensor_tensor(
                        out=xT[:, t, sl], in0=ps,
                        scalar=bo[:, t:t + 1], in1=xT[:, t, sl],
                        op0=OP.add, op1=OP.add,
                    )

            def emit_op(l, c, t):
                emit_op_partial(l, c, t, 0)
                emit_op_finish(l, c, t, 0)

            # LN pieces
            def ln_alloc(nm):
                st1 = ln_ps.tile([1, SC], F32, tag="st1", name=f"st1{nm}")
                st2 = ln_ps.tile([1, SC], F32, tag="st2", name=f"st2{nm}")
                return st1, st2

            def ln_stats_di(st1, st2, c, di):
                sl = slice(c * SC, (c + 1) * SC)
                sq = roll.tile([128, SC], BF16, tag="sq")
                nc.scalar.activation(out=sq, in_=xT[:, di, sl], func=AF.Square)
                nc.tensor.matmul(
                    st1, lhsT=ones_col, rhs=xT[:, di, sl],
                    start=(di == 0), stop=(di == ND - 1),
                )
                nc.tensor.matmul(
                    st2, lhsT=ones_col, rhs=sq,
                    start=(di == 0), stop=(di == ND - 1),
                )

            def ln_stats(st1, st2, c):
                for di in range(ND):
                    ln_stats_di(st1, st2, c, di)

            def ln_rows_bcast(st1, st2, abt, bbt, c):
                """Broadcast the raw per-token sums to all partitions first
                (PE), then do mean/var/sqrt/reciprocal on [128, SC] tiles —
                single-partition DVE math (esp. reciprocal) is ~6x slower."""
                r1 = lnp.tile([1, SC], BF16, tag="r1")
                r2 = lnp.tile([1, SC], BF16, tag="r2")
                nc.scalar.activation(out=r1, in_=st1, func=AF.Identity)
                nc.scalar.activation(out=r2, in_=st2, func=AF.Identity)
                bc1 = ln_bc.tile([128, SC], F32, tag="bc")
                nc.tensor.matmul(bc1, lhsT=ones_row, rhs=r1,
                                 start=True, stop=True)
                bc2 = ln_bc.tile([128, SC], F32, tag="bc")
                nc.tensor.matmul(bc2, lhsT=ones_row, rhs=r2,
                                 start=True, stop=True)
                mw = lnp.tile([128, SC], F32, tag="mw")
                nc.vector.tensor_scalar(
                    out=mw, in0=bc1, scalar1=1.0 / D, scalar2=None, op0=OP.mult,
                )
                m2w = lnp.tile([128, SC], F32, tag="m2w")
                nc.vector.tensor_mul(m2w, mw, mw)
                vw = lnp.tile([128, SC], F32, tag="vw")
                nc.vector.scalar_tensor_tensor(
                    out=vw, in0=bc2, scalar=1.0 / D, in1=m2w,
                    op0=OP.mult, op1=OP.subtract,
                )
                nc.scalar.activation(
                    out=vw, in_=vw, func=AF.Sqrt, bias=epst[:, 0:1],
                )
                with nc.allow_low_precision(reason="bf16 rstd within tolerance"):
                    nc.vector.reciprocal(abt[:, c, :], vw)
                with nc.allow_low_precision(reason="bf16 LN shift"):
                    nc.vector.scalar_tensor_tensor(
                        out=bbt[:, c, :], in0=mw, scalar=-1.0,
                        in1=abt[:, c, :], op0=OP.mult, op1=OP.mult,
                    )

            def ln_apply_di(abt, bbt, gt, ht, c, di, x8out=False):
                sl = slice(c * SC, (c + 1) * SC)
                d = xT[:, di, sl]
                nc.vector.tensor_mul(d, d, abt[:, c, :])
                nc.vector.tensor_add(d, d, bbt[:, c, :])
                if not skip_affine:
                    nc.vector.tensor_scalar(
                        out=d, in0=d,
                        scalar1=gt[:, di:di + 1], scalar2=ht[:, di:di + 1],
                        op0=OP.mult, op1=OP.add,
                    )
                if x8out:
                    with nc.allow_low_precision(reason="fp8 gemm operand"):
                        nc.scalar.activation(
                            out=x8[:, di, sl], in_=d, func=AF.Identity,
                        )

            def ln_apply(abt, bbt, gt, ht, c, x8out=False):
                for di in range(ND):
                    ln_apply_di(abt, bbt, gt, ht, c, di, x8out)

            def emit_w1(l, c, ft_lo, ft_hi):
                sl = slice(c * SC, (c + 1) * SC)
                b1 = biases[l]["b1"]
                for ft in range(ft_lo, ft_hi):
                    w1 = w1p.tile([128, ND, 128], BF16, tag="w1",
                                  name=f"w1_{l}_{c}_{ft}")
                    nc.sync.dma_start(out=w1, in_=w1t[l, ft])
                    ps = p_mm.tile([128, SC], F32, tag="mm")
                    for di in range(ND):
                        nc.tensor.matmul(
                            ps, lhsT=w1[:, di, :], rhs=xT[:, di, sl],
                            start=(di == 0), stop=(di == ND - 1),
                        )
                    nc.scalar.activation(
                        out=hT[0][:, ft, :], in_=ps, func=AF.Relu,
                        bias=b1[:, ft:ft + 1],
                    )

            hT = [None]

            def emit_w2(l, c, st12=None, post=None):
                sl = slice(c * SC, (c + 1) * SC)
                b2 = biases[l]["b2"]
                for t in range(ND):
                    if st12 is not None and t >= 1:
                        ln_stats_di(st12[0], st12[1], c, t - 1)
                    w2 = wpool.tile([128, NF, 128], BF16, tag="w2",
                                    name=f"w2_{l}_{c}_{t}", bufs=3)
                    nc.sync.dma_start(out=w2, in_=w2t[l, t])
                    ps = p_mm.tile([128, SC], F32, tag="mm")
                    for ft in range(NF):
                        nc.tensor.matmul(
                            ps, lhsT=w2[:, ft, :], rhs=hT[0][:, ft, :],
                            start=(ft == 0), stop=(ft == NF - 1),
                        )
                    with nc.allow_low_precision(reason="bf16 residual"):
                        nc.vector.scalar_tensor_tensor(
                            out=xT[:, t, sl], in0=ps,
                            scalar=b2[:, t:t + 1], in1=xT[:, t, sl],
                            op0=OP.add, op1=OP.add,
                        )
                    if post is not None:
                        post(t)
                if st12 is not None:
                    ln_stats_di(st12[0], st12[1], c, ND - 1)

            wh = cst.tile([128, ND, NOUT], BF16, tag="wh")
            nc.sync.dma_start(out=wh, in_=wht[:, :, :])

            def emit_head(st_lo, st_hi):
                for st in range(st_lo, st_hi):
                    ps = p_mm.tile([128, NOUT], F32, tag="mm")
                    for di in range(ND):
                        nc.tensor.matmul(
                            ps, lhsT=xT[:, di, st * 128:(st + 1) * 128],
                            rhs=wh[:, di, :],
                            start=(di == 0), stop=(di == ND - 1),
                        )
                    ob = roll.tile([128, NOUT], F32, tag="ob")
                    nc.vector.tensor_add(ob, ps, bhbt)
                    nc.sync.dma_start(out=y[st * 128:(st + 1) * 128, :], in_=ob)

            def tap(name, ap):
                if taps is None or name not in taps:
                    return
                t = nc.dram_tensor(f"tap_{name}", list(ap.shape), ap.dtype,
                                   kind="ExternalOutput")
                sl = tuple(slice(0, d) for d in ap.shape)
                nc.sync.dma_start(out=t[sl], in_=ap)

            # ================= main flow =================

            # layer-0 entry (no boundary cover needed)
            emit_v(0, 0, NT)
            emit_qk(0, 0, (0, 1))
            emit_qk(0, 1, (0, 1))
            emit_qk(0, 2, (0, 1))
            tap("vT", vT)
            tap("qb00", qks[(0, 0)][2])
            tap("kp00", qks[(0, 0)][3])

            for l in range(L):
                # ---- attention phase
                with tc.tile_pool(name="p_sc", bufs=2, space="PSUM") as p_sc, \
                     tc.tile_pool(name="p_av", bufs=2, space="PSUM") as p_av, \
                     tc.tile_pool(name="p_tr", bufs=1, space="PSUM") as p_tr:
                    emit_scores(l, 0, p_sc)
                    if l == 0:
                        tap("ex0", sms[(0, 0)][0])
                        tap("ex3", sms[(0, 0)][3])
                    for h in range(1, H):
                        if h + 2 < H:
                            emit_qk(l, h + 2, (0, 1))
                        emit_scores(l, h, p_sc)
                        if h == H - 2:
                            emit_op_partial(l, 0, 0, 5)
                        elif h == H - 1:
                            emit_op_partial(l, 0, 1, 6)
                        emit_av(l, h - 1, p_av, p_tr)
                    emit_op_partial(l, 0, 2, 7)
                    emit_av(l, H - 1, p_av, p_tr)
                    # filler while the last head's normalize/copy chain
                    # drains on vector: heads 0..4 of the remaining out-proj
                    # tiles are ready; park them in the draining attention
                    # PSUM pools (p_mm's 3 bufs are all held).
                    emit_op_partial(l, 0, 3, 5, pool=p_sc, tag="sc")
                    emit_op_partial(l, 0, 4, 5, pool=p_sc, tag="sc")
                    emit_op_partial(l, 0, 5, 5, pool=p_av, tag="av")
                    emit_op_partial(l, 0, 6, 5, pool=p_av, tag="av")
                    emit_op_partial(l, 0, 7, 5, pool=p_tr, tag="tr")
                    emit_op_finish(l, 0, 0, 5)
                    emit_op_finish(l, 0, 1, 6)
                    emit_op_finish(l, 0, 2, 7)
                    for t in range(3, ND):
                        emit_op_finish(l, 0, t, 5)

                if l == 0:
                    tap("oT0", oT)
                    tap("xT0", xT)
                # ---- LN1 + FFN + LN2 (+ next-layer V/QK or head as cover)
                with tc.tile_pool(name="ln_ps", bufs=1, space="PSUM") as ln_ps, \
                     tc.tile_pool(name="ln_bc", bufs=2, space="PSUM") as ln_bc:
                    gb = biases[l]
                    abt = lnp.tile([128, NSC, SC], BF16, tag="abt")
                    bbt = lnp.tile([128, NSC, SC], BF16, tag="bbt")

                    st10, st20 = ln_alloc(f"a{l}0")
                    for t in range(4):
                        emit_op(l, 1, t)
                        ln_stats_di(st10, st20, 0, 2 * t)
                        ln_stats_di(st10, st20, 0, 2 * t + 1)
                    emit_op(l, 1, 4)
                    ln_rows_bcast(st10, st20, abt, bbt, 0)
                    emit_op(l, 1, 5)
                    emit_op(l, 1, 6)
                    emit_op(l, 1, 7)
                    ln_apply(abt, bbt, gb["g1"], gb["h1"], 0)
                    st11, st21 = ln_alloc(f"a{l}1")
                    ln_stats(st11, st21, 1)
                    hT[0] = hpool.tile([128, NF, SC], BF16, tag="hT",
                                       name=f"hT{l}_0")
                    emit_w1(l, 0, 0, 4)
                    ln_rows_bcast(st11, st21, abt, bbt, 1)
                    ln_apply(abt, bbt, gb["g1"], gb["h1"], 1)
                    emit_w1(l, 0, 4, NF)
                    if l + 1 < L:
                        load_biases(l + 1)
                    st1b, st2b = ln_alloc(f"b{l}0")
                    emit_w2(l, 0, st12=(st1b, st2b))
                    hT[0] = hpool.tile([128, NF, SC], BF16, tag="hT",
                                       name=f"hT{l}_1")
                    emit_w1(l, 1, 0, 16)
                    ln_rows_bcast(st1b, st2b, abt, bbt, 0)
                    emit_w1(l, 1, 16, NF)
                    if l + 1 < L:
                        load_wv(l + 1)
                    st1c, st2c = ln_alloc(f"b{l}1")
                    # chunk-0 LN2 apply only touches xT chunk 0; W2 chunk 1
                    # reads hT and writes chunk 1, so hide the apply per-di
                    # between W2 tiles (emitting it all up front starves the
                    # W2 stts behind it on the vector queue).
                    emit_w2(l, 1, st12=(st1c, st2c),
                            post=lambda t: ln_apply_di(
                                abt, bbt, gb["g2"], gb["h2"], 0, t,
                                x8out=(l + 1 < L)))
                    if l + 1 < L:
                        emit_v(l + 1, 0, 2)
                        emit_qk(l + 1, 0, (0,))
                        ln_rows_bcast(st1c, st2c, abt, bbt, 1)
                        emit_v(l + 1, 2, 4)
                        emit_qk(l + 1, 1, (0,))
                        emit_qk(l + 1, 2, (0,))
                        ln_apply(abt, bbt, gb["g2"], gb["h2"], 1, x8out=True)
                        emit_v(l + 1, 4, NT)
                        emit_qk(l + 1, 0, (1,))
                        emit_qk(l + 1, 1, (1,))
                        emit_qk(l + 1, 2, (1,))
                    else:
                        emit_head(0, 2)
                        ln_rows_bcast(st1c, st2c, abt, bbt, 1)
                        emit_head(2, 4)
                        ln_apply(abt, bbt, gb["g2"], gb["h2"], 1)
                        emit_head(4, NT)

    _split_multi_waits(nc)
    return nc


def _host_prep(inputs):
    """Pre-transpose / quantize weights, fold pos into X. Layout work."""
    import ml_dtypes
    bf = ml_dtypes.bfloat16
    e4 = ml_dtypes.float8_e4m3
    f32 = np.float32
    inp = {k: np.asarray(v, dtype=f32) for k, v in inputs.items()}

    pos = np.arange(S, dtype=f32)[:, None]
    div = np.exp(np.arange(0, D, 2, dtype=f32) * (-np.log(10000.0) / D)).astype(f32)
    pe = np.zeros((S, D), f32)
    pe[:, 0::2] = np.sin(pos * div)
    pe[:, 1::2] = np.cos(pos * div)

    # band mask: key tile partitions j'=0..127, query cols q'=0..158 (+pad)
    jj = np.arange(128)[:, None]
    qq = np.arange(160)[None, :]
    band01 = (((qq - jj) >= 0) & ((qq - jj) < WIN)).astype(f32)

    def colmajor(v):  # [L, X] -> [L, 128, X/128] col slices
        return np.ascontiguousarray(v.reshape(L, -1, 128).transpose(0, 2, 1))

    def lhsT_tiles(wT, n_out):
        # wT: [L, K, M] (w^T); -> [L, n_out, 128, K/128, 128] where
        # [l, t, p, ki, m] = wT[l, 128*ki + p, 128*t + m]
        Lw, Kw, Mw = wT.shape
        assert Mw == n_out * 128
        r = wT.reshape(Lw, Kw // 128, 128, n_out, 128)
        return np.ascontiguousarray(r.transpose(0, 3, 2, 1, 4))

    def wscale(w):
        rms = np.sqrt(np.mean(w.astype(np.float64) ** 2))
        return float(2.0 ** np.round(np.log2(2.0 / max(rms, 1e-30))))

    # fp8 DoubleRow tiles: pairs of K-tiles (2kt, 2kt+1) on free dim 1
    def qk_tiles8(wT, s):  # wT [D, D] -> [H, 128, KT2, 2, 128]
        r = (wT * s).reshape(KT2, 2, 128, H, 128)
        return np.ascontiguousarray(r.transpose(3, 2, 0, 1, 4)).astype(e4)

    def v_tiles8(wT, s):  # wT [D, D] -> [NSC, 128, KT2, 2, SC]
        r = (wT * s).reshape(KT2, 2, 128, NSC, SC)
        return np.ascontiguousarray(r.transpose(3, 2, 0, 1, 4)).astype(e4)

    wqk8 = np.empty((L, 2, H, 128, KT2, 2, 128), e4)
    wv8 = np.empty((L, NSC, 128, KT2, 2, SC), e4)
    dqqv = np.empty((L, 128, 3), f32)
    for l in range(L):
        sq = wscale(inp["Wq"][l])
        sk = wscale(inp["Wk"][l])
        sv = wscale(inp["Wv"][l])
        wqk8[l, 0] = qk_tiles8(inp["Wq"][l].T, sq)
        wqk8[l, 1] = qk_tiles8(inp["Wk"][l].T, sk)
        wv8[l] = v_tiles8(inp["Wv"][l].T, sv)
        dqqv[l, :, 0] = SCALE / sq
        dqqv[l, :, 1] = 1.0 / sk
        dqqv[l, :, 2] = 1.0 / sv

    skip_affine = bool(
        np.all(inp["ln1_g"] == 1.0) and np.all(inp["ln1_b"] == 0.0)
        and np.all(inp["ln2_g"] == 1.0) and np.all(inp["ln2_b"] == 0.0)
    )

    shared = {
        "band": band01.astype(bf),
        "ident": np.eye(128, dtype=f32).astype(bf),
        "wqk8": wqk8,
        "wv8": wv8,
        "dqqv": dqqv,
        "wot": lhsT_tiles(inp["Wo"].transpose(0, 2, 1), ND).astype(bf),
        "w1t": lhsT_tiles(inp["W1"].transpose(0, 2, 1), NF).astype(bf),
        "w2t": lhsT_tiles(inp["W2"].transpose(0, 2, 1), ND).astype(bf),
        "wht": np.ascontiguousarray(
            inp["Wh"].T.reshape(ND, 128, NOUT).transpose(1, 0, 2)).astype(bf),
        "bqs": colmajor(inp["bq"] * SCALE),
        "bkc": colmajor(inp["bk"]),
        "bvb": inp["bv"].reshape(L, 1, D).astype(bf),
        "boc": colmajor(inp["bo"]),
        "b1c": colmajor(inp["b1"]),
        "b2c": colmajor(inp["b2"]),
        "g1c": colmajor(inp["ln1_g"]),
        "h1c": colmajor(inp["ln1_b"]),
        "g2c": colmajor(inp["ln2_g"]),
        "h2c": colmajor(inp["ln2_b"]),
        "onesc": np.ones((128, 1), f32).astype(bf),
        "onesr": np.ones((1, 128), f32).astype(bf),
        "bhb": np.ascontiguousarray(inp["bh"].reshape(1, NOUT)),
    }
    in_maps = []
    for b in range(N_CORES):
        xb = (inp["X"][b] + pe).T  # [D, S]
        m = dict(shared)
        m["xt16"] = np.ascontiguousarray(xb).astype(bf)
        m["xt8"] = np.ascontiguousarray(
            xb.reshape(ND, 128, S).transpose(1, 0, 2)).astype(e4)
        in_maps.append(m)
    return in_maps, skip_affine


_NC_CACHE = {}


def run(inputs, trace=False, **spmd_kwargs):
    in_maps, skip_affine = _host_prep(inputs)
    key = ("nc", skip_affine)
    if key not in _NC_CACHE:
        _NC_CACHE[key] = build_program(skip_affine=skip_affine)
    nc = _NC_CACHE[key]
    res = run_bass_kernel_spmd(
        nc, in_maps, list(range(N_CORES)), trace=trace, **spmd_kwargs
    )
    out = np.concatenate([res.results[i]["out"] for i in range(N_CORES)], axis=0)
    return out, res


def kernel(**inputs) -> np.ndarray:
    out, _ = run(inputs, trace=False)
    return out


# revision 7
# speedup vs baseline: 1.0302x; 1.0199x over previous
"""LocalWindowTransformer Trainium2 kernel, v3.

Data-parallel over batch (B=8 -> 8 cores). bf16 datapath with fp8
(e4m3 DoubleRow) QKV projections: K=1024 contractions run as 4
pair-packed K=256 matmuls at 2x PE throughput (weights pre-scaled to
e4m3 range on host, dequant folded into the PSUM-drain activations).
Attention is computed in transposed-score form: scores land
[keys, queries] (one matmul per 128-key tile over its 159-query band,
window=32), exp'd and band-masked, and the AV matmul consumes them
directly; the AV rhs carries a ones column so the softmax denominator
falls out of the same matmul, and a per-tile reciprocal+scale
normalizes before one transpose back into head-major oT. LN row stats
accumulate via ones-column matmuls interleaved with the GEMM streams;
the row math runs on broadcast [128,SC] tiles, with rstd computed as
exp(-0.5*ln(var+eps)) on the scalar engine — one act-table set covers
every scalar function used, and the 3.3us DVE reciprocal disappears. Weight pools are deep enough
(w1p=6, w2/wo multi-buf) to keep the PE from draining its DVFS ramp.
PSUM accumulation and LN row stats stay f32.
"""

import numpy as np

import concourse.bass as bass
import concourse.tile as tile
from concourse import mybir
from concourse.bass_utils import run_bass_kernel_spmd
from concourse.vector_clock import ScopedClock

F32 = mybir.dt.float32
BF16 = mybir.dt.bfloat16
FP8 = mybir.dt.float8e4
DR = mybir.MatmulPerfMode.DoubleRow
AF = mybir.ActivationFunctionType
OP = mybir.AluOpType

B, S, D = 8, 1024, 1024
H, HD, L, FF, NOUT, WIN = 8, 128, 4, 4096, 512, 32
ND = D // 128     # 8 feature tiles
NT = S // 128     # 8 token tiles
NF = FF // 128    # 32 ffn tiles
KT2 = ND // 2     # 4 feature-pair tiles (fp8 DoubleRow)
SC = 512          # s-chunk
NSC = S // SC     # 2
NQB = 159         # banded query cols per key tile (128 + WIN - 1)
SCALE = 1.0 / float(np.sqrt(HD))
EPS = 1e-5
N_CORES = 8


class SafeTileContext(tile.TileContext):
    """walrus in this image rejects a Drain carrying several sem waits
    ("Too many sync wait commands"). Absorb the outstanding waits into
    single-wait SP nops before the tail drain so the drain itself is
    wait-free."""

    def _drain_and_barrier(self, tick_clock, wait_clock):
        gclock = tick_clock.global_clock
        for proc in range(len(gclock)):
            tick = gclock[proc]
            if tick > 0:
                partial = ScopedClock()
                partial.require_at_least(None, proc, tick)
                nop = self.nc.sync.nop(nofuse=True)
                wait_clock.add_sem_waits(nop.ins, partial)
        self.nc.sync.drain()
        self.nc.all_engine_barrier()
        popped = self.nc._tile_sem_poison_stack.pop()
        assert popped is self._sem_poison
        self.nc.clear_and_free_semaphores(list(self.sems.allocated().values()))
        self.nc.all_engine_barrier()


def _split_multi_waits(nc):
    """This image's walrus accepts at most ONE sem wait per instruction.
    Hoist extra waits onto same-engine NoOps immediately preceding the
    instruction."""
    n = 0
    for f in nc.m.functions:
        for bb in f.blocks:
            insts = bb.instructions
            out = []
            for inst in insts:
                si = inst.sync_info
                waits = list(si.on_wait) if si is not None else []
                if len(waits) > 1:
                    for w in waits[:-1]:
                        n += 1
                        nop = mybir.InstNoOp(
                            name=f"{inst.name}-wsplit{n}",
                            engine=inst.engine,
                            ins=[], outs=[],
                            sync_info=mybir.SyncInfo(on_wait=[w], on_update=[]),
                        )
                        out.append(nop)
                    inst.sync_info = mybir.SyncInfo(
                        on_wait=[waits[-1]], on_update=list(si.on_update)
                    )
                out.append(inst)
            insts[:] = out
    return n


def build_program(skip_affine=False, taps=None):
    nc = bass.Bass()

    xt16 = nc.dram_tensor("xt16", [D, S], BF16, kind="ExternalInput")
    xt8 = nc.dram_tensor("xt8", [128, ND, S], FP8, kind="ExternalInput")
    band = nc.dram_tensor("band", [128, 160], BF16, kind="ExternalInput")
    ident = nc.dram_tensor("ident", [128, 128], BF16, kind="ExternalInput")
    wqk8 = nc.dram_tensor("wqk8", [L, 2, H, 128, KT2, 2, 128], FP8,
                          kind="ExternalInput")
    wv8 = nc.dram_tensor("wv8", [L, NSC, 128, KT2, 2, SC], FP8,
                         kind="ExternalInput")
    wot = nc.dram_tensor("wot", [L, ND, 128, ND, 128], BF16, kind="ExternalInput")
    w1t = nc.dram_tensor("w1t", [L, NF, 128, ND, 128], BF16, kind="ExternalInput")
    w2t = nc.dram_tensor("w2t", [L, ND, 128, NF, 128], BF16, kind="ExternalInput")
    wht = nc.dram_tensor("wht", [128, ND, NOUT], BF16, kind="ExternalInput")
    bqs = nc.dram_tensor("bqs", [L, 128, H], F32, kind="ExternalInput")  # bq*SCALE
    bkc = nc.dram_tensor("bkc", [L, 128, H], F32, kind="ExternalInput")
    bvb = nc.dram_tensor("bvb", [L, 1, D], BF16, kind="ExternalInput")
    dqqv = nc.dram_tensor("dqqv", [L, 128, 3], F32, kind="ExternalInput")
    boc = nc.dram_tensor("boc", [L, 128, ND], F32, kind="ExternalInput")
    b1c = nc.dram_tensor("b1c", [L, 128, NF], F32, kind="ExternalInput")
    b2c = nc.dram_tensor("b2c", [L, 128, ND], F32, kind="ExternalInput")
    g1c = nc.dram_tensor("g1c", [L, 128, ND], F32, kind="ExternalInput")
    h1c = nc.dram_tensor("h1c", [L, 128, ND], F32, kind="ExternalInput")
    g2c = nc.dram_tensor("g2c", [L, 128, ND], F32, kind="ExternalInput")
    h2c = nc.dram_tensor("h2c", [L, 128, ND], F32, kind="ExternalInput")
    onesc = nc.dram_tensor("onesc", [128, 1], BF16, kind="ExternalInput")
    onesr = nc.dram_tensor("onesr", [1, 128], BF16, kind="ExternalInput")
    bhb = nc.dram_tensor("bhb", [1, NOUT], F32, kind="ExternalInput")
    y = nc.dram_tensor("out", [S, NOUT], F32, kind="ExternalOutput")

    with SafeTileContext(nc) as tc:
        from contextlib import ExitStack

        with ExitStack() as ctx:
            ep = ctx.enter_context
            p_mm = ep(tc.tile_pool(name="p_mm", bufs=3, space="PSUM"))
            wpool = ep(tc.tile_pool(name="wpool", bufs=2))
            wsm = ep(tc.tile_pool(name="wsm", bufs=2))
            w1p = ep(tc.tile_pool(name="w1p", bufs=6))
            big = ep(tc.tile_pool(name="big", bufs=1))
            hpool = ep(tc.tile_pool(name="hpool", bufs=1))
            qkp = ep(tc.tile_pool(name="qkp", bufs=3))
            smp = ep(tc.tile_pool(name="smp", bufs=20))
            cst = ep(tc.tile_pool(name="cst", bufs=1))
            bias = ep(tc.tile_pool(name="bias", bufs=2))
            lnp = ep(tc.tile_pool(name="lnp", bufs=1))
            roll = ep(tc.tile_pool(name="roll", bufs=2))

            # ---- input + layer-0 weights first so compute starts ASAP
            x8 = big.tile([128, ND, S], FP8, tag="x8")
            for kt in range(KT2):
                nc.sync.dma_start(
                    out=x8[:, 2 * kt:2 * kt + 2, :],
                    in_=xt8[:, 2 * kt:2 * kt + 2, :],
                )

            biases = {}

            def load_biases(l):
                t = {}
                for nm, src, shape, dt in (
                    ("bq", bqs, [128, H], F32), ("bk", bkc, [128, H], F32),
                    ("bo", boc, [128, ND], F32), ("b1", b1c, [128, NF], F32),
                    ("b2", b2c, [128, ND], F32), ("g1", g1c, [128, ND], F32),
                    ("h1", h1c, [128, ND], F32), ("g2", g2c, [128, ND], F32),
                    ("h2", h2c, [128, ND], F32),
                ):
                    tl = bias.tile(shape, dt, tag=nm, name=f"{nm}_{l}")
                    nc.sync.dma_start(out=tl, in_=src[l])
                    t[nm] = tl
                dq = bias.tile([128, 3], F32, tag="dq", name=f"dq_{l}")
                nc.sync.dma_start(out=dq, in_=dqqv[l])
                t["dqq"], t["dqk"], t["dqv"] = dq[:, 0:1], dq[:, 1:2], dq[:, 2:3]
                bvt = bias.tile([128, ND, 128], BF16, tag="bv", name=f"bv_{l}")
                nc.sync.dma_start(
                    out=bvt, in_=bvb[l].to_broadcast([128, D])
                )
                t["bv"] = bvt
                biases[l] = t

            load_biases(0)

            vtiles = {}

            def load_wv(l):
                wv = []
                for dc in range(NSC):
                    wt = wpool.tile([128, KT2, 2, SC], FP8, tag="wv",
                                    name=f"wv{l}_{dc}")
                    nc.sync.dma_start(out=wt, in_=wv8[l, dc])
                    wv.append(wt)
                vtiles[l] = wv

            load_wv(0)

            # ---- constants (after the hot-path DMAs)
            bandt = cst.tile([128, 160], BF16, tag="bandt")
            nc.sync.dma_start(out=bandt, in_=band[:, :])
            idt = cst.tile([128, 128], BF16, tag="idt")
            nc.sync.dma_start(out=idt, in_=ident[:, :])
            ones_col = cst.tile([128, 1], BF16, tag="onc")
            nc.sync.dma_start(out=ones_col, in_=onesc[:, :])
            ones_row = cst.tile([1, 128], BF16, tag="onr")
            nc.sync.dma_start(out=ones_row, in_=onesr[:, :])
            bhbt = cst.tile([128, NOUT], F32, tag="bhb")
            nc.sync.dma_start(out=bhbt, in_=bhb[0:1, :].to_broadcast([128, NOUT]))
            epst = cst.tile([128, 1], F32, tag="eps")
            nc.vector.memset(epst, EPS)

            xT = big.tile([128, ND, S], BF16, tag="xT")
            for di in range(ND):
                nc.sync.dma_start(
                    out=xT[:, di, :], in_=xt16[di * 128:(di + 1) * 128, :]
                )

            vT = big.tile([128, NT, H, 129], BF16, tag="vT")
            nc.vector.memset(vT[:, :, :, 128:129], 1.0)
            oT = big.tile([128, H, S], BF16, tag="oT")

            # ================= emit helpers =================

            def emit_v(l, st_lo, st_hi):
                wv = vtiles[l]
                bvt = biases[l]["bv"]
                dqv = biases[l]["dqv"]
                for st in range(st_lo, st_hi):
                    for dc in range(NSC):
                        ps = p_mm.tile([128, 4, 128], F32, tag="mm")
                        for kt in range(KT2):
                            nc.tensor.matmul(
                                ps,
                                lhsT=x8[:, 2 * kt:2 * kt + 2,
                                        st * 128:(st + 1) * 128],
                                rhs=wv[dc][:, kt],
                                start=(kt == 0), stop=(kt == KT2 - 1),
                                perf_mode=DR,
                            )
                        with nc.allow_low_precision(reason="bf16 v"):
                            nc.vector.scalar_tensor_tensor(
                                out=vT[:, st, 4 * dc:4 * dc + 4, 0:128],
                                in0=ps, scalar=dqv,
                                in1=bvt[:, 4 * dc:4 * dc + 4, :],
                                op0=OP.mult, op1=OP.add,
                            )

            qks = {}

            def emit_qk(l, h, cs):
                """QK projections for head h, chunks cs (subset of {0,1})."""
                if (l, h) not in qks:
                    wq = wsm.tile([128, KT2, 2, 128], FP8, tag="wq",
                                  name=f"wq{l}_{h}", bufs=3)
                    nc.sync.dma_start(out=wq, in_=wqk8[l, 0, h])
                    wk = wsm.tile([128, KT2, 2, 128], FP8, tag="wk",
                                  name=f"wk{l}_{h}", bufs=3)
                    nc.sync.dma_start(out=wk, in_=wqk8[l, 1, h])
                    qb = qkp.tile([128, S], BF16, tag="qh", name=f"qb{l}_{h}")
                    kp = qkp.tile([128, S], BF16, tag="kh", name=f"kp{l}_{h}")
                    qks[(l, h)] = (wq, wk, qb, kp)
                wq, wk, qb, kp = qks[(l, h)]
                bq, bk = biases[l]["bq"], biases[l]["bk"]
                dqq, dqk = biases[l]["dqq"], biases[l]["dqk"]
                for c in cs:
                    sl = slice(c * SC, (c + 1) * SC)
                    psq = p_mm.tile([128, SC], F32, tag="mm")
                    for kt in range(KT2):
                        nc.tensor.matmul(
                            psq, lhsT=wq[:, kt],
                            rhs=x8[:, 2 * kt:2 * kt + 2, sl],
                            start=(kt == 0), stop=(kt == KT2 - 1),
                            perf_mode=DR,
                        )
                    nc.scalar.activation(
                        out=qb[:, sl], in_=psq, func=AF.Identity,
                        bias=bq[:, h:h + 1], scale=dqq,
                    )
                    psk = p_mm.tile([128, SC], F32, tag="mm")
                    for kt in range(KT2):
                        nc.tensor.matmul(
                            psk, lhsT=wk[:, kt],
                            rhs=x8[:, 2 * kt:2 * kt + 2, sl],
                            start=(kt == 0), stop=(kt == KT2 - 1),
                            perf_mode=DR,
                        )
                    nc.scalar.activation(
                        out=kp[:, sl], in_=psk, func=AF.Identity,
                        bias=bk[:, h:h + 1], scale=dqk,
                    )

            sms = {}

            def emit_scores(l, h, p_sc):
                """Transposed banded scores + exp for all key tiles of head h:
                sc[j', q'] for keys in tile jt, queries jt*128 .. +158."""
                _, _, qb, kp = qks[(l, h)]
                tiles = []
                for jt in range(NT):
                    nq = NQB if jt < NT - 1 else 128
                    scp = p_sc.tile([128, NQB], F32, tag="sc")
                    nc.tensor.matmul(
                        scp[:, 0:nq],
                        lhsT=kp[:, jt * 128:(jt + 1) * 128],
                        rhs=qb[:, jt * 128:jt * 128 + nq],
                        start=True, stop=True,
                    )
                    ex = smp.tile([128, 160], BF16, tag="ex")
                    nc.scalar.activation(
                        out=ex[:, 0:nq], in_=scp[:, 0:nq], func=AF.Exp,
                    )
                    with nc.allow_low_precision(reason="bf16 probs"):
                        nc.vector.tensor_mul(
                            ex[:, 0:nq], ex[:, 0:nq], bandt[:, 0:nq]
                        )
                    tiles.append(ex)
                sms[(l, h)] = tiles

            def emit_av(l, h, p_av, p_tr, mid=None):
                """AV (+denominator via ones column) for all query tiles of
                head h; av comes out [queries, hd+1]: normalize per-partition
                into ot tiles while the PE streams the next its, then
                transpose all tiles back into head-major oT. Transposes share
                the av pool slots (by the time they run, those avs are
                consumed)."""
                tiles = sms.pop((l, h))
                ots = []
                avs = {}

                def av_diag(it):
                    av = p_av.tile([128, 129], F32, tag="av")
                    avs[it] = av
                    nc.tensor.matmul(
                        av, lhsT=tiles[it][:, 0:128],
                        rhs=vT[:, it, h, :],
                        start=True, stop=(it == 0), skip_group_check=True,
                    )

                def av_prev(it):
                    # rows 0..95 of the prev tile's tail cols are band-masked
                    # zeros, so the full partition range contributes only the
                    # window overlap.
                    nc.tensor.matmul(
                        avs[it][0:31, :],
                        lhsT=tiles[it - 1][:, 128:159],
                        rhs=vT[:, it - 1, h, :],
                        start=False, stop=True, skip_group_check=True,
                    )

                def av_norm(it):
                    av = avs.pop(it)
                    rc = smp.tile([128, 1], F32, tag="rc")
                    nc.vector.reciprocal(rc, av[:, 128:129])
                    ot = smp.tile([128, 128], BF16, tag="ot")
                    with nc.allow_low_precision(reason="bf16 o"):
                        nc.vector.tensor_scalar(
                            out=ot, in0=av[:, 0:128],
                            scalar1=rc, scalar2=None, op0=OP.mult,
                        )
                    ots.append(ot)

                # stagger diag/prev so consecutive matmuls never target the
                # same PSUM region back-to-back (write-drain ~173ns).
                av_diag(0)
                av_norm(0)
                av_diag(1)
                for it in range(2, NT):
                    av_diag(it)
                    av_prev(it - 1)
                    av_norm(it - 1)
                av_prev(NT - 1)
                av_norm(NT - 1)
                if mid is not None:
                    mid()
                for it in range(NT):
                    trp = p_tr.tile([128, 128], BF16, tag="tr")
                    nc.tensor.transpose(trp, ots[it], idt)
                    nc.vector.tensor_copy(
                        out=oT[:, h, it * 128:(it + 1) * 128], in_=trp
                    )

            def load_wo(l, t):
                wo = wsm.tile([128, ND, 128], BF16, tag="wo", name=f"wo{l}_{t}",
                              bufs=4)
                nc.sync.dma_start(out=wo, in_=wot[l, t])
                return wo

            op_state = {}

            def emit_op_partial(l, c, t, di_hi, pool=None, tag="mm"):
                wo = load_wo(l, t)
                sl = slice(c * SC, (c + 1) * SC)
                ps = (pool or p_mm).tile([128, SC], F32, tag=tag)
                for di in range(di_hi):
                    nc.tensor.matmul(
                        ps, lhsT=wo[:, di, :], rhs=oT[:, di, sl],
                        start=(di == 0), stop=False,
                    )
                op_state[(c, t)] = (wo, ps)

            def emit_op_finish(l, c, t, di_lo):
                wo, ps = op_state.pop((c, t))
                sl = slice(c * SC, (c + 1) * SC)
                bo = biases[l]["bo"]
                for di in range(di_lo, ND):
                    nc.tensor.matmul(
                        ps, lhsT=wo[:, di, :], rhs=oT[:, di, sl],
                        start=(di == 0), stop=(di == ND - 1),
                    )
                with nc.allow_low_precision(reason="bf16 residual"):
                    nc.vector.scalar_tensor_tensor(
                        out=xT[:, t, sl], in0=ps,
                        scalar=bo[:, t:t + 1], in1=xT[:, t, sl],
                        op0=OP.add, op1=OP.add,
                    )

            def emit_op(l, c, t):
                emit_op_partial(l, c, t, 0)
                emit_op_finish(l, c, t, 0)

            # LN pieces
            def ln_alloc(nm):
                st1 = ln_ps.tile([1, SC], F32, tag="st1", name=f"st1{nm}")
                st2 = ln_ps.tile([1, SC], F32, tag="st2", name=f"st2{nm}")
                return st1, st2

            def ln_stats_di(st1, st2, c, di):
                sl = slice(c * SC, (c + 1) * SC)
                sq = roll.tile([128, SC], BF16, tag="sq")
                nc.scalar.activation(out=sq, in_=xT[:, di, sl], func=AF.Square)
                nc.tensor.matmul(
                    st1, lhsT=ones_col, rhs=xT[:, di, sl],
                    start=(di == 0), stop=(di == ND - 1),
                )
                nc.tensor.matmul(
                    st2, lhsT=ones_col, rhs=sq,
                    start=(di == 0), stop=(di == ND - 1),
                )

            def ln_stats(st1, st2, c):
                for di in range(ND):
                    ln_stats_di(st1, st2, c, di)

            def ln_rows_bcast(st1, st2, abt, bbt, c):
                """Broadcast the raw per-token sums to all partitions first
                (PE), then do mean/var/sqrt/reciprocal on [128, SC] tiles —
                single-partition DVE math (esp. reciprocal) is ~6x slower."""
                r1 = lnp.tile([1, SC], BF16, tag="r1")
                r2 = lnp.tile([1, SC], BF16, tag="r2")
                nc.scalar.activation(out=r1, in_=st1, func=AF.Identity)
                nc.scalar.activation(out=r2, in_=st2, func=AF.Identity)
                bc1 = ln_bc.tile([128, SC], F32, tag="bc")
                nc.tensor.matmul(bc1, lhsT=ones_row, rhs=r1,
                                 start=True, stop=True)
                bc2 = ln_bc.tile([128, SC], F32, tag="bc")
                nc.tensor.matmul(bc2, lhsT=ones_row, rhs=r2,
                                 start=True, stop=True)
                mw = lnp.tile([128, SC], F32, tag="mw")
                nc.vector.tensor_scalar(
                    out=mw, in0=bc1, scalar1=1.0 / D, scalar2=None, op0=OP.mult,
                )
                m2w = lnp.tile([128, SC], F32, tag="m2w")
                nc.vector.tensor_mul(m2w, mw, mw)
                vw = lnp.tile([128, SC], F32, tag="vw")
                nc.vector.scalar_tensor_tensor(
                    out=vw, in0=bc2, scalar=1.0 / D, in1=m2w,
                    op0=OP.mult, op1=OP.subtract,
                )
                # rstd = exp(-0.5*ln(var+eps)): two scalar-table ops in the
                # same act set as exp/relu/square/identity (no table swaps),
                # replacing the scalar sqrt + the 3.3us DVE reciprocal.
                lnv = lnp.tile([128, SC], F32, tag="lnv")
                nc.scalar.activation(
                    out=lnv, in_=vw, func=AF.Ln, bias=epst[:, 0:1],
                )
                with nc.allow_low_precision(reason="bf16 rstd within tolerance"):
                    nc.scalar.activation(
                        out=abt[:, c, :], in_=lnv, func=AF.Exp, scale=-0.5,
                    )
                with nc.allow_low_precision(reason="bf16 LN shift"):
                    nc.vector.scalar_tensor_tensor(
                        out=bbt[:, c, :], in0=mw, scalar=-1.0,
                        in1=abt[:, c, :], op0=OP.mult, op1=OP.mult,
                    )

            def ln_apply_di(abt, bbt, gt, ht, c, di, x8out=False):
                sl = slice(c * SC, (c + 1) * SC)
                d = xT[:, di, sl]
                nc.vector.tensor_mul(d, d, abt[:, c, :])
                nc.vector.tensor_add(d, d, bbt[:, c, :])
                if not skip_affine:
                    nc.vector.tensor_scalar(
                        out=d, in0=d,
                        scalar1=gt[:, di:di + 1], scalar2=ht[:, di:di + 1],
                        op0=OP.mult, op1=OP.add,
                    )
                if x8out:
                    with nc.allow_low_precision(reason="fp8 gemm operand"):
                        nc.scalar.activation(
                            out=x8[:, di, sl], in_=d, func=AF.Identity,
                        )

            def ln_apply(abt, bbt, gt, ht, c, x8out=False):
                for di in range(ND):
                    ln_apply_di(abt, bbt, gt, ht, c, di, x8out)

            def emit_w1(l, c, ft_lo, ft_hi):
                sl = slice(c * SC, (c + 1) * SC)
                b1 = biases[l]["b1"]
                for ft in range(ft_lo, ft_hi):
                    w1 = w1p.tile([128, ND, 128], BF16, tag="w1",
                                  name=f"w1_{l}_{c}_{ft}")
                    nc.sync.dma_start(out=w1, in_=w1t[l, ft])
                    ps = p_mm.tile([128, SC], F32, tag="mm")
                    for di in range(ND):
                        nc.tensor.matmul(
                            ps, lhsT=w1[:, di, :], rhs=xT[:, di, sl],
                            start=(di == 0), stop=(di == ND - 1),
                        )
                    nc.scalar.activation(
                        out=hT[0][:, ft, :], in_=ps, func=AF.Relu,
                        bias=b1[:, ft:ft + 1],
                    )

            hT = [None]

            def emit_w2(l, c, st12=None, post=None):
                sl = slice(c * SC, (c + 1) * SC)
                b2 = biases[l]["b2"]
                for t in range(ND):
                    if st12 is not None and t >= 1:
                        ln_stats_di(st12[0], st12[1], c, t - 1)
                    w2 = wpool.tile([128, NF, 128], BF16, tag="w2",
                                    name=f"w2_{l}_{c}_{t}", bufs=3)
                    nc.sync.dma_start(out=w2, in_=w2t[l, t])
                    ps = p_mm.tile([128, SC], F32, tag="mm")
                    for ft in range(NF):
                        nc.tensor.matmul(
                            ps, lhsT=w2[:, ft, :], rhs=hT[0][:, ft, :],
                            start=(ft == 0), stop=(ft == NF - 1),
                        )
                    with nc.allow_low_precision(reason="bf16 residual"):
                        nc.vector.scalar_tensor_tensor(
                            out=xT[:, t, sl], in0=ps,
                            scalar=b2[:, t:t + 1], in1=xT[:, t, sl],
                            op0=OP.add, op1=OP.add,
                        )
                    if post is not None:
                        post(t)
                if st12 is not None:
                    ln_stats_di(st12[0], st12[1], c, ND - 1)

            wh = cst.tile([128, ND, NOUT], BF16, tag="wh")
            nc.sync.dma_start(out=wh, in_=wht[:, :, :])

            def emit_head(st_lo, st_hi):
                for st in range(st_lo, st_hi):
                    ps = p_mm.tile([128, NOUT], F32, tag="mm")
                    for di in range(ND):
                        nc.tensor.matmul(
                            ps, lhsT=xT[:, di, st * 128:(st + 1) * 128],
                            rhs=wh[:, di, :],
                            start=(di == 0), stop=(di == ND - 1),
                        )
                    ob = roll.tile([128, NOUT], F32, tag="ob")
                    nc.vector.tensor_add(ob, ps, bhbt)
                    nc.sync.dma_start(out=y[st * 128:(st + 1) * 128, :], in_=ob)

            def tap(name, ap):
                if taps is None or name not in taps:
                    return
                t = nc.dram_tensor(f"tap_{name}", list(ap.shape), ap.dtype,
                                   kind="ExternalOutput")
                sl = tuple(slice(0, d) for d in ap.shape)
                nc.sync.dma_start(out=t[sl], in_=ap)

            # ================= main flow =================

            # layer-0 entry (no boundary cover needed)
            emit_v(0, 0, NT)
            emit_qk(0, 0, (0, 1))
            emit_qk(0, 1, (0, 1))
            emit_qk(0, 2, (0, 1))
            tap("vT", vT)
            tap("qb00", qks[(0, 0)][2])
            tap("kp00", qks[(0, 0)][3])

            for l in range(L):
                # ---- attention phase
                with tc.tile_pool(name="p_sc", bufs=2, space="PSUM") as p_sc, \
                     tc.tile_pool(name="p_av", bufs=2, space="PSUM") as p_av, \
                     tc.tile_pool(name="p_tr", bufs=1, space="PSUM") as p_tr:
                    emit_scores(l, 0, p_sc)
                    if l == 0:
                        tap("ex0", sms[(0, 0)][0])
                        tap("ex3", sms[(0, 0)][3])
                    for h in range(1, H):
                        if h + 2 < H:
                            emit_qk(l, h + 2, (0, 1))
                        emit_scores(l, h, p_sc)
                        if h == H - 2:
                            emit_op_partial(l, 0, 0, 5)
                        elif h == H - 1:
                            emit_op_partial(l, 0, 1, 6)
                        emit_av(l, h - 1, p_av, p_tr)
                    emit_op_partial(l, 0, 2, 7)
                    emit_av(l, H - 1, p_av, p_tr)
                    # filler while the last head's normalize/copy chain
                    # drains on vector: heads 0..4 of the remaining out-proj
                    # tiles are ready; park them in the draining attention
                    # PSUM pools (p_mm's 3 bufs are all held).
                    emit_op_partial(l, 0, 3, 5, pool=p_sc, tag="sc")
                    emit_op_partial(l, 0, 4, 5, pool=p_sc, tag="sc")
                    emit_op_partial(l, 0, 5, 5, pool=p_av, tag="av")
                    emit_op_partial(l, 0, 6, 5, pool=p_av, tag="av")
                    emit_op_partial(l, 0, 7, 5, pool=p_tr, tag="tr")
                    emit_op_finish(l, 0, 0, 5)
                    emit_op_finish(l, 0, 1, 6)
                    emit_op_finish(l, 0, 2, 7)
                    for t in range(3, ND):
                        emit_op_finish(l, 0, t, 5)

                if l == 0:
                    tap("oT0", oT)
                    tap("xT0", xT)
                # ---- LN1 + FFN + LN2 (+ next-layer V/QK or head as cover)
                with tc.tile_pool(name="ln_ps", bufs=1, space="PSUM") as ln_ps, \
                     tc.tile_pool(name="ln_bc", bufs=2, space="PSUM") as ln_bc:
                    gb = biases[l]
                    abt = lnp.tile([128, NSC, SC], BF16, tag="abt")
                    bbt = lnp.tile([128, NSC, SC], BF16, tag="bbt")

                    st10, st20 = ln_alloc(f"a{l}0")
                    for t in range(4):
                        emit_op(l, 1, t)
                        ln_stats_di(st10, st20, 0, 2 * t)
                        ln_stats_di(st10, st20, 0, 2 * t + 1)
                    emit_op(l, 1, 4)
                    ln_rows_bcast(st10, st20, abt, bbt, 0)
                    emit_op(l, 1, 5)
                    emit_op(l, 1, 6)
                    emit_op(l, 1, 7)
                    ln_apply(abt, bbt, gb["g1"], gb["h1"], 0)
                    st11, st21 = ln_alloc(f"a{l}1")
                    ln_stats(st11, st21, 1)
                    hT[0] = hpool.tile([128, NF, SC], BF16, tag="hT",
                                       name=f"hT{l}_0")
                    emit_w1(l, 0, 0, 4)
                    ln_rows_bcast(st11, st21, abt, bbt, 1)
                    ln_apply(abt, bbt, gb["g1"], gb["h1"], 1)
                    emit_w1(l, 0, 4, NF)
                    if l + 1 < L:
                        load_biases(l + 1)
                    st1b, st2b = ln_alloc(f"b{l}0")
                    emit_w2(l, 0, st12=(st1b, st2b))
                    hT[0] = hpool.tile([128, NF, SC], BF16, tag="hT",
                                       name=f"hT{l}_1")
                    emit_w1(l, 1, 0, 16)
                    ln_rows_bcast(st1b, st2b, abt, bbt, 0)
                    emit_w1(l, 1, 16, NF)
                    if l + 1 < L:
                        load_wv(l + 1)
                    st1c, st2c = ln_alloc(f"b{l}1")
                    # chunk-0 LN2 apply only touches xT chunk 0; W2 chunk 1
                    # reads hT and writes chunk 1, so hide the apply per-di
                    # between W2 tiles (emitting it all up front starves the
                    # W2 stts behind it on the vector queue).
                    emit_w2(l, 1, st12=(st1c, st2c),
                            post=lambda t: ln_apply_di(
                                abt, bbt, gb["g2"], gb["h2"], 0, t,
                                x8out=(l + 1 < L)))
                    if l + 1 < L:
                        emit_v(l + 1, 0, 2)
                        emit_qk(l + 1, 0, (0,))
                        ln_rows_bcast(st1c, st2c, abt, bbt, 1)
                        emit_v(l + 1, 2, 4)
                        emit_qk(l + 1, 1, (0,))
                        emit_qk(l + 1, 2, (0,))
                        ln_apply(abt, bbt, gb["g2"], gb["h2"], 1, x8out=True)
                        emit_v(l + 1, 4, NT)
                        emit_qk(l + 1, 0, (1,))
                        emit_qk(l + 1, 1, (1,))
                        emit_qk(l + 1, 2, (1,))
                    else:
                        emit_head(0, 2)
                        ln_rows_bcast(st1c, st2c, abt, bbt, 1)
                        emit_head(2, 4)
                        ln_apply(abt, bbt, gb["g2"], gb["h2"], 1)
                        emit_head(4, NT)

    _split_multi_waits(nc)
    return nc


def _host_prep(inputs):
    """Pre-transpose / quantize weights, fold pos into X. Layout work."""
    import ml_dtypes
    bf = ml_dtypes.bfloat16
    e4 = ml_dtypes.float8_e4m3
    f32 = np.float32
    inp = {k: np.asarray(v, dtype=f32) for k, v in inputs.items()}

    pos = np.arange(S, dtype=f32)[:, None]
    div = np.exp(np.arange(0, D, 2, dtype=f32) * (-np.log(10000.0) / D)).astype(f32)
    pe = np.zeros((S, D), f32)
    pe[:, 0::2] = np.sin(pos * div)
    pe[:, 1::2] = np.cos(pos * div)

    # band mask: key tile partitions j'=0..127, query cols q'=0..158 (+pad)
    jj = np.arange(128)[:, None]
    qq = np.arange(160)[None, :]
    band01 = (((qq - jj) >= 0) & ((qq - jj) < WIN)).astype(f32)

    def colmajor(v):  # [L, X] -> [L, 128, X/128] col slices
        return np.ascontiguousarray(v.reshape(L, -1, 128).transpose(0, 2, 1))

    def lhsT_tiles(wT, n_out):
        # wT: [L, K, M] (w^T); -> [L, n_out, 128, K/128, 128] where
        # [l, t, p, ki, m] = wT[l, 128*ki + p, 128*t + m]
        Lw, Kw, Mw = wT.shape
        assert Mw == n_out * 128
        r = wT.reshape(Lw, Kw // 128, 128, n_out, 128)
        return np.ascontiguousarray(r.transpose(0, 3, 2, 1, 4))

    def wscale(w):
        rms = np.sqrt(np.mean(w.astype(np.float64) ** 2))
        return float(2.0 ** np.round(np.log2(2.0 / max(rms, 1e-30))))

    # fp8 DoubleRow tiles: pairs of K-tiles (2kt, 2kt+1) on free dim 1
    def qk_tiles8(wT, s):  # wT [D, D] -> [H, 128, KT2, 2, 128]
        r = (wT * s).reshape(KT2, 2, 128, H, 128)
        return np.ascontiguousarray(r.transpose(3, 2, 0, 1, 4)).astype(e4)

    def v_tiles8(wT, s):  # wT [D, D] -> [NSC, 128, KT2, 2, SC]
        r = (wT * s).reshape(KT2, 2, 128, NSC, SC)
        return np.ascontiguousarray(r.transpose(3, 2, 0, 1, 4)).astype(e4)

    wqk8 = np.empty((L, 2, H, 128, KT2, 2, 128), e4)
    wv8 = np.empty((L, NSC, 128, KT2, 2, SC), e4)
    dqqv = np.empty((L, 128, 3), f32)
    for l in range(L):
        sq = wscale(inp["Wq"][l])
        sk = wscale(inp["Wk"][l])
        sv = wscale(inp["Wv"][l])
        wqk8[l, 0] = qk_tiles8(inp["Wq"][l].T, sq)
        wqk8[l, 1] = qk_tiles8(inp["Wk"][l].T, sk)
        wv8[l] = v_tiles8(inp["Wv"][l].T, sv)
        dqqv[l, :, 0] = SCALE / sq
        dqqv[l, :, 1] = 1.0 / sk
        dqqv[l, :, 2] = 1.0 / sv

    skip_affine = bool(
        np.all(inp["ln1_g"] == 1.0) and np.all(inp["ln1_b"] == 0.0)
        and np.all(inp["ln2_g"] == 1.0) and np.all(inp["ln2_b"] == 0.0)
    )

    shared = {
        "band": band01.astype(bf),
        "ident": np.eye(128, dtype=f32).astype(bf),
        "wqk8": wqk8,
        "wv8": wv8,
        "dqqv": dqqv,
        "wot": lhsT_tiles(inp["Wo"].transpose(0, 2, 1), ND).astype(bf),
        "w1t": lhsT_tiles(inp["W1"].transpose(0, 2, 1), NF).astype(bf),
        "w2t": lhsT_tiles(inp["W2"].transpose(0, 2, 1), ND).astype(bf),
        "wht": np.ascontiguousarray(
            inp["Wh"].T.reshape(ND, 128, NOUT).transpose(1, 0, 2)).astype(bf),
        "bqs": colmajor(inp["bq"] * SCALE),
        "bkc": colmajor(inp["bk"]),
        "bvb": inp["bv"].reshape(L, 1, D).astype(bf),
        "boc": colmajor(inp["bo"]),
        "b1c": colmajor(inp["b1"]),
        "b2c": colmajor(inp["b2"]),
        "g1c": colmajor(inp["ln1_g"]),
        "h1c": colmajor(inp["ln1_b"]),
        "g2c": colmajor(inp["ln2_g"]),
        "h2c": colmajor(inp["ln2_b"]),
        "onesc": np.ones((128, 1), f32).astype(bf),
        "onesr": np.ones((1, 128), f32).astype(bf),
        "bhb": np.ascontiguousarray(inp["bh"].reshape(1, NOUT)),
    }
    in_maps = []
    for b in range(N_CORES):
        xb = (inp["X"][b] + pe).T  # [D, S]
        m = dict(shared)
        m["xt16"] = np.ascontiguousarray(xb).astype(bf)
        m["xt8"] = np.ascontiguousarray(
            xb.reshape(ND, 128, S).transpose(1, 0, 2)).astype(e4)
        in_maps.append(m)
    return in_maps, skip_affine


_NC_CACHE = {}


def run(inputs, trace=False, **spmd_kwargs):
    in_maps, skip_affine = _host_prep(inputs)
    key = ("nc", skip_affine)
    if key not in _NC_CACHE:
        _NC_CACHE[key] = build_program(skip_affine=skip_affine)
    nc = _NC_CACHE[key]
    res = run_bass_kernel_spmd(
        nc, in_maps, list(range(N_CORES)), trace=trace, **spmd_kwargs
    )
    out = np.concatenate([res.results[i]["out"] for i in range(N_CORES)], axis=0)
    return out, res


def kernel(**inputs) -> np.ndarray:
    out, _ = run(inputs, trace=False)
    return out


# revision 8
# speedup vs baseline: 1.0310x; 1.0008x over previous
"""LocalWindowTransformer Trainium2 kernel, v3.

Data-parallel over batch (B=8 -> 8 cores). bf16 datapath with fp8
(e4m3 DoubleRow) QKV projections: K=1024 contractions run as 4
pair-packed K=256 matmuls at 2x PE throughput (weights pre-scaled to
e4m3 range on host, dequant folded into the PSUM-drain activations).
Attention is computed in transposed-score form: scores land
[keys, queries] (one matmul per 128-key tile over its 159-query band,
window=32), exp'd and band-masked, and the AV matmul consumes them
directly; the AV rhs carries a ones column so the softmax denominator
falls out of the same matmul, and a per-tile reciprocal+scale
normalizes before one transpose back into head-major oT. LN row stats
accumulate via ones-column matmuls interleaved with the GEMM streams;
the row math runs on broadcast [128,SC] tiles, with rstd computed as
exp(-0.5*ln(var+eps)) on the scalar engine — one act-table set covers
every scalar function used, and the 3.3us DVE reciprocal disappears. Weight pools are deep enough
(w1p=6, w2/wo multi-buf) to keep the PE from draining its DVFS ramp.
PSUM accumulation and LN row stats stay f32.
"""

import numpy as np

import concourse.bass as bass
import concourse.tile as tile
from concourse import mybir
from concourse.bass_utils import run_bass_kernel_spmd
from concourse.vector_clock import ScopedClock

F32 = mybir.dt.float32
BF16 = mybir.dt.bfloat16
FP8 = mybir.dt.float8e4
DR = mybir.MatmulPerfMode.DoubleRow
AF = mybir.ActivationFunctionType
OP = mybir.AluOpType

B, S, D = 8, 1024, 1024
H, HD, L, FF, NOUT, WIN = 8, 128, 4, 4096, 512, 32
ND = D // 128     # 8 feature tiles
NT = S // 128     # 8 token tiles
NF = FF // 128    # 32 ffn tiles
KT2 = ND // 2     # 4 feature-pair tiles (fp8 DoubleRow)
SC = 512          # s-chunk
NSC = S // SC     # 2
NQB = 159         # banded query cols per key tile (128 + WIN - 1)
SCALE = 1.0 / float(np.sqrt(HD))
EPS = 1e-5
N_CORES = 8


class SafeTileContext(tile.TileContext):
    """walrus in this image rejects a Drain carrying several sem waits
    ("Too many sync wait commands"). Absorb the outstanding waits into
    single-wait SP nops before the tail drain so the drain itself is
    wait-free."""

    def _drain_and_barrier(self, tick_clock, wait_clock):
        gclock = tick_clock.global_clock
        for proc in range(len(gclock)):
            tick = gclock[proc]
            if tick > 0:
                partial = ScopedClock()
                partial.require_at_least(None, proc, tick)
                nop = self.nc.sync.nop(nofuse=True)
                wait_clock.add_sem_waits(nop.ins, partial)
        self.nc.sync.drain()
        self.nc.all_engine_barrier()
        popped = self.nc._tile_sem_poison_stack.pop()
        assert popped is self._sem_poison
        self.nc.clear_and_free_semaphores(list(self.sems.allocated().values()))
        self.nc.all_engine_barrier()


def _split_multi_waits(nc):
    """This image's walrus accepts at most ONE sem wait per instruction.
    Hoist extra waits onto same-engine NoOps immediately preceding the
    instruction."""
    n = 0
    for f in nc.m.functions:
        for bb in f.blocks:
            insts = bb.instructions
            out = []
            for inst in insts:
                si = inst.sync_info
                waits = list(si.on_wait) if si is not None else []
                if len(waits) > 1:
                    for w in waits[:-1]:
                        n += 1
                        nop = mybir.InstNoOp(
                            name=f"{inst.name}-wsplit{n}",
                            engine=inst.engine,
                            ins=[], outs=[],
                            sync_info=mybir.SyncInfo(on_wait=[w], on_update=[]),
                        )
                        out.append(nop)
                    inst.sync_info = mybir.SyncInfo(
                        on_wait=[waits[-1]], on_update=list(si.on_update)
                    )
                out.append(inst)
            insts[:] = out
    return n


def build_program(skip_affine=False, taps=None):
    nc = bass.Bass()

    xt16 = nc.dram_tensor("xt16", [D, S], BF16, kind="ExternalInput")
    xt8 = nc.dram_tensor("xt8", [128, ND, S], FP8, kind="ExternalInput")
    band = nc.dram_tensor("band", [128, 160], BF16, kind="ExternalInput")
    ident = nc.dram_tensor("ident", [128, 128], BF16, kind="ExternalInput")
    wqk8 = nc.dram_tensor("wqk8", [L, 2, H, 128, KT2, 2, 128], FP8,
                          kind="ExternalInput")
    wv8 = nc.dram_tensor("wv8", [L, NSC, 128, KT2, 2, SC], FP8,
                         kind="ExternalInput")
    wot = nc.dram_tensor("wot", [L, ND, 128, ND, 128], BF16, kind="ExternalInput")
    w1t = nc.dram_tensor("w1t", [L, NF, 128, ND, 128], BF16, kind="ExternalInput")
    w2t = nc.dram_tensor("w2t", [L, ND, 128, NF, 128], BF16, kind="ExternalInput")
    wht = nc.dram_tensor("wht", [128, ND, NOUT], BF16, kind="ExternalInput")
    bqs = nc.dram_tensor("bqs", [L, 128, H], F32, kind="ExternalInput")  # bq*SCALE
    bkc = nc.dram_tensor("bkc", [L, 128, H], F32, kind="ExternalInput")
    bvb = nc.dram_tensor("bvb", [L, 1, D], BF16, kind="ExternalInput")
    dqqv = nc.dram_tensor("dqqv", [L, 128, 3], F32, kind="ExternalInput")
    boc = nc.dram_tensor("boc", [L, 128, ND], F32, kind="ExternalInput")
    b1c = nc.dram_tensor("b1c", [L, 128, NF], F32, kind="ExternalInput")
    b2c = nc.dram_tensor("b2c", [L, 128, ND], F32, kind="ExternalInput")
    g1c = nc.dram_tensor("g1c", [L, 128, ND], F32, kind="ExternalInput")
    h1c = nc.dram_tensor("h1c", [L, 128, ND], F32, kind="ExternalInput")
    g2c = nc.dram_tensor("g2c", [L, 128, ND], F32, kind="ExternalInput")
    h2c = nc.dram_tensor("h2c", [L, 128, ND], F32, kind="ExternalInput")
    onesc = nc.dram_tensor("onesc", [128, 1], BF16, kind="ExternalInput")
    onesr = nc.dram_tensor("onesr", [1, 128], BF16, kind="ExternalInput")
    bhb = nc.dram_tensor("bhb", [1, NOUT], F32, kind="ExternalInput")
    y = nc.dram_tensor("out", [S, NOUT], F32, kind="ExternalOutput")

    with SafeTileContext(nc) as tc:
        from contextlib import ExitStack

        with ExitStack() as ctx:
            ep = ctx.enter_context
            p_mm = ep(tc.tile_pool(name="p_mm", bufs=3, space="PSUM"))
            wpool = ep(tc.tile_pool(name="wpool", bufs=2))
            wsm = ep(tc.tile_pool(name="wsm", bufs=2))
            w1p = ep(tc.tile_pool(name="w1p", bufs=6))
            big = ep(tc.tile_pool(name="big", bufs=1))
            hpool = ep(tc.tile_pool(name="hpool", bufs=1))
            qkp = ep(tc.tile_pool(name="qkp", bufs=3))
            smp = ep(tc.tile_pool(name="smp", bufs=16))
            cst = ep(tc.tile_pool(name="cst", bufs=1))
            bias = ep(tc.tile_pool(name="bias", bufs=2))
            lnp = ep(tc.tile_pool(name="lnp", bufs=1))
            roll = ep(tc.tile_pool(name="roll", bufs=2))

            # ---- input + layer-0 weights first so compute starts ASAP
            x8 = big.tile([128, ND, S], FP8, tag="x8")
            for kt in range(KT2):
                nc.sync.dma_start(
                    out=x8[:, 2 * kt:2 * kt + 2, :],
                    in_=xt8[:, 2 * kt:2 * kt + 2, :],
                )

            biases = {}

            def load_biases(l):
                t = {}
                for nm, src, shape, dt in (
                    ("bq", bqs, [128, H], F32), ("bk", bkc, [128, H], F32),
                    ("bo", boc, [128, ND], F32), ("b1", b1c, [128, NF], F32),
                    ("b2", b2c, [128, ND], F32), ("g1", g1c, [128, ND], F32),
                    ("h1", h1c, [128, ND], F32), ("g2", g2c, [128, ND], F32),
                    ("h2", h2c, [128, ND], F32),
                ):
                    tl = bias.tile(shape, dt, tag=nm, name=f"{nm}_{l}")
                    nc.sync.dma_start(out=tl, in_=src[l])
                    t[nm] = tl
                dq = bias.tile([128, 3], F32, tag="dq", name=f"dq_{l}")
                nc.sync.dma_start(out=dq, in_=dqqv[l])
                t["dqq"], t["dqk"], t["dqv"] = dq[:, 0:1], dq[:, 1:2], dq[:, 2:3]
                bvt = bias.tile([128, ND, 128], BF16, tag="bv", name=f"bv_{l}")
                nc.sync.dma_start(
                    out=bvt, in_=bvb[l].to_broadcast([128, D])
                )
                t["bv"] = bvt
                biases[l] = t

            load_biases(0)

            vtiles = {}

            def load_wv(l):
                wv = []
                for dc in range(NSC):
                    wt = wpool.tile([128, KT2, 2, SC], FP8, tag="wv",
                                    name=f"wv{l}_{dc}")
                    nc.sync.dma_start(out=wt, in_=wv8[l, dc])
                    wv.append(wt)
                vtiles[l] = wv

            load_wv(0)

            # ---- constants (after the hot-path DMAs)
            bandt = cst.tile([128, 160], BF16, tag="bandt")
            nc.sync.dma_start(out=bandt, in_=band[:, :])
            idt = cst.tile([128, 128], BF16, tag="idt")
            nc.sync.dma_start(out=idt, in_=ident[:, :])
            ones_col = cst.tile([128, 1], BF16, tag="onc")
            nc.sync.dma_start(out=ones_col, in_=onesc[:, :])
            ones_row = cst.tile([1, 128], BF16, tag="onr")
            nc.sync.dma_start(out=ones_row, in_=onesr[:, :])
            bhbt = cst.tile([128, NOUT], F32, tag="bhb")
            nc.sync.dma_start(out=bhbt, in_=bhb[0:1, :].to_broadcast([128, NOUT]))
            epst = cst.tile([128, 1], F32, tag="eps")
            nc.vector.memset(epst, EPS)

            xT = big.tile([128, ND, S], BF16, tag="xT")
            for di in range(ND):
                nc.sync.dma_start(
                    out=xT[:, di, :], in_=xt16[di * 128:(di + 1) * 128, :]
                )

            vT = big.tile([128, NT, H, 129], BF16, tag="vT")
            nc.vector.memset(vT[:, :, :, 128:129], 1.0)
            # pre-zero the ex-tile tails (cols 159:256) across all pool slots;
            # nothing ever writes there, so the zeros persist through reuse
            # and let the prev-AV matmul run full-width (M=31 partial tiles
            # cost ~160ns vs ~105ns for M=128).
            for _i in range(16):
                _ez = smp.tile([128, 256], BF16, tag="ex", name=f"exz{_i}")
                nc.vector.memset(_ez[:, 159:256], 0.0)
            oT = big.tile([128, H, S], BF16, tag="oT")

            # ================= emit helpers =================

            def emit_v(l, st_lo, st_hi):
                wv = vtiles[l]
                bvt = biases[l]["bv"]
                dqv = biases[l]["dqv"]
                for st in range(st_lo, st_hi):
                    for dc in range(NSC):
                        ps = p_mm.tile([128, 4, 128], F32, tag="mm")
                        for kt in range(KT2):
                            nc.tensor.matmul(
                                ps,
                                lhsT=x8[:, 2 * kt:2 * kt + 2,
                                        st * 128:(st + 1) * 128],
                                rhs=wv[dc][:, kt],
                                start=(kt == 0), stop=(kt == KT2 - 1),
                                perf_mode=DR,
                            )
                        with nc.allow_low_precision(reason="bf16 v"):
                            nc.vector.scalar_tensor_tensor(
                                out=vT[:, st, 4 * dc:4 * dc + 4, 0:128],
                                in0=ps, scalar=dqv,
                                in1=bvt[:, 4 * dc:4 * dc + 4, :],
                                op0=OP.mult, op1=OP.add,
                            )

            qks = {}

            def emit_qk(l, h, cs):
                """QK projections for head h, chunks cs (subset of {0,1})."""
                if (l, h) not in qks:
                    wq = wsm.tile([128, KT2, 2, 128], FP8, tag="wq",
                                  name=f"wq{l}_{h}", bufs=3)
                    nc.sync.dma_start(out=wq, in_=wqk8[l, 0, h])
                    wk = wsm.tile([128, KT2, 2, 128], FP8, tag="wk",
                                  name=f"wk{l}_{h}", bufs=3)
                    nc.sync.dma_start(out=wk, in_=wqk8[l, 1, h])
                    qb = qkp.tile([128, S], BF16, tag="qh", name=f"qb{l}_{h}")
                    kp = qkp.tile([128, S], BF16, tag="kh", name=f"kp{l}_{h}")
                    qks[(l, h)] = (wq, wk, qb, kp)
                wq, wk, qb, kp = qks[(l, h)]
                bq, bk = biases[l]["bq"], biases[l]["bk"]
                dqq, dqk = biases[l]["dqq"], biases[l]["dqk"]
                for c in cs:
                    sl = slice(c * SC, (c + 1) * SC)
                    psq = p_mm.tile([128, SC], F32, tag="mm")
                    for kt in range(KT2):
                        nc.tensor.matmul(
                            psq, lhsT=wq[:, kt],
                            rhs=x8[:, 2 * kt:2 * kt + 2, sl],
                            start=(kt == 0), stop=(kt == KT2 - 1),
                            perf_mode=DR,
                        )
                    nc.scalar.activation(
                        out=qb[:, sl], in_=psq, func=AF.Identity,
                        bias=bq[:, h:h + 1], scale=dqq,
                    )
                    psk = p_mm.tile([128, SC], F32, tag="mm")
                    for kt in range(KT2):
                        nc.tensor.matmul(
                            psk, lhsT=wk[:, kt],
                            rhs=x8[:, 2 * kt:2 * kt + 2, sl],
                            start=(kt == 0), stop=(kt == KT2 - 1),
                            perf_mode=DR,
                        )
                    nc.scalar.activation(
                        out=kp[:, sl], in_=psk, func=AF.Identity,
                        bias=bk[:, h:h + 1], scale=dqk,
                    )

            sms = {}

            def emit_scores(l, h, p_sc):
                """Transposed banded scores + exp for all key tiles of head h:
                sc[j', q'] for keys in tile jt, queries jt*128 .. +158."""
                _, _, qb, kp = qks[(l, h)]
                tiles = []
                for jt in range(NT):
                    nq = NQB if jt < NT - 1 else 128
                    scp = p_sc.tile([128, NQB], F32, tag="sc")
                    nc.tensor.matmul(
                        scp[:, 0:nq],
                        lhsT=kp[:, jt * 128:(jt + 1) * 128],
                        rhs=qb[:, jt * 128:jt * 128 + nq],
                        start=True, stop=True,
                    )
                    ex = smp.tile([128, 256], BF16, tag="ex")
                    nc.scalar.activation(
                        out=ex[:, 0:nq], in_=scp[:, 0:nq], func=AF.Exp,
                    )
                    with nc.allow_low_precision(reason="bf16 probs"):
                        nc.vector.tensor_mul(
                            ex[:, 0:nq], ex[:, 0:nq], bandt[:, 0:nq]
                        )
                    tiles.append(ex)
                sms[(l, h)] = tiles

            def emit_av(l, h, p_av, p_tr, mid=None):
                """AV (+denominator via ones column) for all query tiles of
                head h; av comes out [queries, hd+1]: normalize per-partition
                into ot tiles while the PE streams the next its, then
                transpose all tiles back into head-major oT. Transposes share
                the av pool slots (by the time they run, those avs are
                consumed)."""
                tiles = sms.pop((l, h))
                ots = []
                avs = {}

                def av_diag(it):
                    av = p_av.tile([128, 129], F32, tag="av")
                    avs[it] = av
                    nc.tensor.matmul(
                        av, lhsT=tiles[it][:, 0:128],
                        rhs=vT[:, it, h, :],
                        start=True, stop=(it == 0), skip_group_check=True,
                    )

                def av_prev(it):
                    # cols 128:159 hold the window-overlap probs; 159:256 are
                    # persistent zeros, so the full-width matmul adds exactly
                    # the prev-tile contribution to queries 0..30.
                    nc.tensor.matmul(
                        avs[it],
                        lhsT=tiles[it - 1][:, 128:256],
                        rhs=vT[:, it - 1, h, :],
                        start=False, stop=True, skip_group_check=True,
                    )

                def av_norm(it):
                    av = avs.pop(it)
                    rc = smp.tile([128, 1], F32, tag="rc")
                    nc.vector.reciprocal(rc, av[:, 128:129])
                    ot = smp.tile([128, 128], BF16, tag="ot")
                    with nc.allow_low_precision(reason="bf16 o"):
                        nc.vector.tensor_scalar(
                            out=ot, in0=av[:, 0:128],
                            scalar1=rc, scalar2=None, op0=OP.mult,
                        )
                    ots.append(ot)

                # stagger diag/prev so consecutive matmuls never target the
                # same PSUM region back-to-back (write-drain ~173ns).
                av_diag(0)
                av_norm(0)
                av_diag(1)
                for it in range(2, NT):
                    av_diag(it)
                    av_prev(it - 1)
                    av_norm(it - 1)
                av_prev(NT - 1)
                av_norm(NT - 1)
                if mid is not None:
                    mid()
                for it in range(NT):
                    trp = p_tr.tile([128, 128], BF16, tag="tr")
                    nc.tensor.transpose(trp, ots[it], idt)
                    nc.vector.tensor_copy(
                        out=oT[:, h, it * 128:(it + 1) * 128], in_=trp
                    )

            def load_wo(l, t):
                wo = wsm.tile([128, ND, 128], BF16, tag="wo", name=f"wo{l}_{t}",
                              bufs=4)
                nc.sync.dma_start(out=wo, in_=wot[l, t])
                return wo

            op_state = {}

            def emit_op_partial(l, c, t, di_hi, pool=None, tag="mm"):
                wo = load_wo(l, t)
                sl = slice(c * SC, (c + 1) * SC)
                ps = (pool or p_mm).tile([128, SC], F32, tag=tag)
                for di in range(di_hi):
                    nc.tensor.matmul(
                        ps, lhsT=wo[:, di, :], rhs=oT[:, di, sl],
                        start=(di == 0), stop=False,
                    )
                op_state[(c, t)] = (wo, ps)

            def emit_op_finish(l, c, t, di_lo):
                wo, ps = op_state.pop((c, t))
                sl = slice(c * SC, (c + 1) * SC)
                bo = biases[l]["bo"]
                for di in range(di_lo, ND):
                    nc.tensor.matmul(
                        ps, lhsT=wo[:, di, :], rhs=oT[:, di, sl],
                        start=(di == 0), stop=(di == ND - 1),
                    )
                with nc.allow_low_precision(reason="bf16 residual"):
                    nc.vector.scalar_tensor_tensor(
                        out=xT[:, t, sl], in0=ps,
                        scalar=bo[:, t:t + 1], in1=xT[:, t, sl],
                        op0=OP.add, op1=OP.add,
                    )

            def emit_op(l, c, t):
                emit_op_partial(l, c, t, 0)
                emit_op_finish(l, c, t, 0)

            # LN pieces
            def ln_alloc(nm):
                st1 = ln_ps.tile([1, SC], F32, tag="st1", name=f"st1{nm}")
                st2 = ln_ps.tile([1, SC], F32, tag="st2", name=f"st2{nm}")
                return st1, st2

            def ln_stats_di(st1, st2, c, di):
                sl = slice(c * SC, (c + 1) * SC)
                sq = roll.tile([128, SC], BF16, tag="sq")
                nc.scalar.activation(out=sq, in_=xT[:, di, sl], func=AF.Square)
                nc.tensor.matmul(
                    st1, lhsT=ones_col, rhs=xT[:, di, sl],
                    start=(di == 0), stop=(di == ND - 1),
                )
                nc.tensor.matmul(
                    st2, lhsT=ones_col, rhs=sq,
                    start=(di == 0), stop=(di == ND - 1),
                )

            def ln_stats(st1, st2, c):
                for di in range(ND):
                    ln_stats_di(st1, st2, c, di)

            def ln_rows_bcast(st1, st2, abt, bbt, c):
                """Broadcast the raw per-token sums to all partitions first
                (PE), then do mean/var/sqrt/reciprocal on [128, SC] tiles —
                single-partition DVE math (esp. reciprocal) is ~6x slower."""
                r1 = lnp.tile([1, SC], BF16, tag="r1")
                r2 = lnp.tile([1, SC], BF16, tag="r2")
                nc.scalar.activation(out=r1, in_=st1, func=AF.Identity)
                nc.scalar.activation(out=r2, in_=st2, func=AF.Identity)
                bc1 = ln_bc.tile([128, SC], F32, tag="bc")
                nc.tensor.matmul(bc1, lhsT=ones_row, rhs=r1,
                                 start=True, stop=True)
                bc2 = ln_bc.tile([128, SC], F32, tag="bc")
                nc.tensor.matmul(bc2, lhsT=ones_row, rhs=r2,
                                 start=True, stop=True)
                mw = lnp.tile([128, SC], F32, tag="mw")
                nc.vector.tensor_scalar(
                    out=mw, in0=bc1, scalar1=1.0 / D, scalar2=None, op0=OP.mult,
                )
                m2w = lnp.tile([128, SC], F32, tag="m2w")
                nc.vector.tensor_mul(m2w, mw, mw)
                vw = lnp.tile([128, SC], F32, tag="vw")
                nc.vector.scalar_tensor_tensor(
                    out=vw, in0=bc2, scalar=1.0 / D, in1=m2w,
                    op0=OP.mult, op1=OP.subtract,
                )
                # rstd = exp(-0.5*ln(var+eps)): two scalar-table ops in the
                # same act set as exp/relu/square/identity (no table swaps),
                # replacing the scalar sqrt + the 3.3us DVE reciprocal.
                lnv = lnp.tile([128, SC], F32, tag="lnv")
                nc.scalar.activation(
                    out=lnv, in_=vw, func=AF.Ln, bias=epst[:, 0:1],
                )
                with nc.allow_low_precision(reason="bf16 rstd within tolerance"):
                    nc.scalar.activation(
                        out=abt[:, c, :], in_=lnv, func=AF.Exp, scale=-0.5,
                    )
                with nc.allow_low_precision(reason="bf16 LN shift"):
                    nc.vector.scalar_tensor_tensor(
                        out=bbt[:, c, :], in0=mw, scalar=-1.0,
                        in1=abt[:, c, :], op0=OP.mult, op1=OP.mult,
                    )

            def ln_apply_di(abt, bbt, gt, ht, c, di, x8out=False):
                sl = slice(c * SC, (c + 1) * SC)
                d = xT[:, di, sl]
                nc.vector.tensor_mul(d, d, abt[:, c, :])
                nc.vector.tensor_add(d, d, bbt[:, c, :])
                if not skip_affine:
                    nc.vector.tensor_scalar(
                        out=d, in0=d,
                        scalar1=gt[:, di:di + 1], scalar2=ht[:, di:di + 1],
                        op0=OP.mult, op1=OP.add,
                    )
                if x8out:
                    with nc.allow_low_precision(reason="fp8 gemm operand"):
                        nc.scalar.activation(
                            out=x8[:, di, sl], in_=d, func=AF.Identity,
                        )

            def ln_apply(abt, bbt, gt, ht, c, x8out=False):
                for di in range(ND):
                    ln_apply_di(abt, bbt, gt, ht, c, di, x8out)

            def emit_w1(l, c, ft_lo, ft_hi):
                sl = slice(c * SC, (c + 1) * SC)
                b1 = biases[l]["b1"]
                for ft in range(ft_lo, ft_hi):
                    w1 = w1p.tile([128, ND, 128], BF16, tag="w1",
                                  name=f"w1_{l}_{c}_{ft}")
                    nc.sync.dma_start(out=w1, in_=w1t[l, ft])
                    ps = p_mm.tile([128, SC], F32, tag="mm")
                    for di in range(ND):
                        nc.tensor.matmul(
                            ps, lhsT=w1[:, di, :], rhs=xT[:, di, sl],
                            start=(di == 0), stop=(di == ND - 1),
                        )
                    nc.scalar.activation(
                        out=hT[0][:, ft, :], in_=ps, func=AF.Relu,
                        bias=b1[:, ft:ft + 1],
                    )

            hT = [None]

            def emit_w2(l, c, st12=None, post=None):
                sl = slice(c * SC, (c + 1) * SC)
                b2 = biases[l]["b2"]
                for t in range(ND):
                    if st12 is not None and t >= 1:
                        ln_stats_di(st12[0], st12[1], c, t - 1)
                    w2 = wpool.tile([128, NF, 128], BF16, tag="w2",
                                    name=f"w2_{l}_{c}_{t}", bufs=3)
                    nc.sync.dma_start(out=w2, in_=w2t[l, t])
                    ps = p_mm.tile([128, SC], F32, tag="mm")
                    for ft in range(NF):
                        nc.tensor.matmul(
                            ps, lhsT=w2[:, ft, :], rhs=hT[0][:, ft, :],
                            start=(ft == 0), stop=(ft == NF - 1),
                        )
                    with nc.allow_low_precision(reason="bf16 residual"):
                        nc.vector.scalar_tensor_tensor(
                            out=xT[:, t, sl], in0=ps,
                            scalar=b2[:, t:t + 1], in1=xT[:, t, sl],
                            op0=OP.add, op1=OP.add,
                        )
                    if post is not None:
                        post(t)
                if st12 is not None:
                    ln_stats_di(st12[0], st12[1], c, ND - 1)

            wh = cst.tile([128, ND, NOUT], BF16, tag="wh")
            nc.sync.dma_start(out=wh, in_=wht[:, :, :])

            def emit_head(st_lo, st_hi):
                for st in range(st_lo, st_hi):
                    ps = p_mm.tile([128, NOUT], F32, tag="mm")
                    for di in range(ND):
                        nc.tensor.matmul(
                            ps, lhsT=xT[:, di, st * 128:(st + 1) * 128],
                            rhs=wh[:, di, :],
                            start=(di == 0), stop=(di == ND - 1),
                        )
                    ob = roll.tile([128, NOUT], F32, tag="ob")
                    nc.vector.tensor_add(ob, ps, bhbt)
                    nc.sync.dma_start(out=y[st * 128:(st + 1) * 128, :], in_=ob)

            def tap(name, ap):
                if taps is None or name not in taps:
                    return
                t = nc.dram_tensor(f"tap_{name}", list(ap.shape), ap.dtype,
                                   kind="ExternalOutput")
                sl = tuple(slice(0, d) for d in ap.shape)
                nc.sync.dma_start(out=t[sl], in_=ap)

            # ================= main flow =================

            # layer-0 entry (no boundary cover needed)
            emit_v(0, 0, NT)
            emit_qk(0, 0, (0, 1))
            emit_qk(0, 1, (0, 1))
            emit_qk(0, 2, (0, 1))
            tap("vT", vT)
            tap("qb00", qks[(0, 0)][2])
            tap("kp00", qks[(0, 0)][3])

            for l in range(L):
                # ---- attention phase
                with tc.tile_pool(name="p_sc", bufs=2, space="PSUM") as p_sc, \
                     tc.tile_pool(name="p_av", bufs=2, space="PSUM") as p_av, \
                     tc.tile_pool(name="p_tr", bufs=1, space="PSUM") as p_tr:
                    emit_scores(l, 0, p_sc)
                    if l == 0:
                        tap("ex0", sms[(0, 0)][0])
                        tap("ex3", sms[(0, 0)][3])
                    for h in range(1, H):
                        if h + 2 < H:
                            emit_qk(l, h + 2, (0, 1))
                        emit_scores(l, h, p_sc)
                        if h == H - 2:
                            emit_op_partial(l, 0, 0, 5)
                        elif h == H - 1:
                            emit_op_partial(l, 0, 1, 6)
                        emit_av(l, h - 1, p_av, p_tr)
                    emit_op_partial(l, 0, 2, 7)
                    emit_av(l, H - 1, p_av, p_tr)
                    # filler while the last head's normalize/copy chain
                    # drains on vector: heads 0..4 of the remaining out-proj
                    # tiles are ready; park them in the draining attention
                    # PSUM pools (p_mm's 3 bufs are all held).
                    emit_op_partial(l, 0, 3, 5, pool=p_sc, tag="sc")
                    emit_op_partial(l, 0, 4, 5, pool=p_sc, tag="sc")
                    emit_op_partial(l, 0, 5, 5, pool=p_av, tag="av")
                    emit_op_partial(l, 0, 6, 5, pool=p_av, tag="av")
                    emit_op_partial(l, 0, 7, 5, pool=p_tr, tag="tr")
                    emit_op_finish(l, 0, 0, 5)
                    emit_op_finish(l, 0, 1, 6)
                    emit_op_finish(l, 0, 2, 7)
                    for t in range(3, ND):
                        emit_op_finish(l, 0, t, 5)

                if l == 0:
                    tap("oT0", oT)
                    tap("xT0", xT)
                # ---- LN1 + FFN + LN2 (+ next-layer V/QK or head as cover)
                with tc.tile_pool(name="ln_ps", bufs=1, space="PSUM") as ln_ps, \
                     tc.tile_pool(name="ln_bc", bufs=2, space="PSUM") as ln_bc:
                    gb = biases[l]
                    abt = lnp.tile([128, NSC, SC], BF16, tag="abt")
                    bbt = lnp.tile([128, NSC, SC], BF16, tag="bbt")

                    st10, st20 = ln_alloc(f"a{l}0")
                    for t in range(4):
                        emit_op(l, 1, t)
                        ln_stats_di(st10, st20, 0, 2 * t)
                        ln_stats_di(st10, st20, 0, 2 * t + 1)
                    emit_op(l, 1, 4)
                    ln_rows_bcast(st10, st20, abt, bbt, 0)
                    emit_op(l, 1, 5)
                    emit_op(l, 1, 6)
                    emit_op(l, 1, 7)
                    ln_apply(abt, bbt, gb["g1"], gb["h1"], 0)
                    st11, st21 = ln_alloc(f"a{l}1")
                    ln_stats(st11, st21, 1)
                    hT[0] = hpool.tile([128, NF, SC], BF16, tag="hT",
                                       name=f"hT{l}_0")
                    emit_w1(l, 0, 0, 4)
                    ln_rows_bcast(st11, st21, abt, bbt, 1)
                    ln_apply(abt, bbt, gb["g1"], gb["h1"], 1)
                    emit_w1(l, 0, 4, NF)
                    if l + 1 < L:
                        load_biases(l + 1)
                    st1b, st2b = ln_alloc(f"b{l}0")
                    emit_w2(l, 0, st12=(st1b, st2b))
                    hT[0] = hpool.tile([128, NF, SC], BF16, tag="hT",
                                       name=f"hT{l}_1")
                    emit_w1(l, 1, 0, 16)
                    ln_rows_bcast(st1b, st2b, abt, bbt, 0)
                    emit_w1(l, 1, 16, NF)
                    if l + 1 < L:
                        load_wv(l + 1)
                    st1c, st2c = ln_alloc(f"b{l}1")
                    # chunk-0 LN2 apply only touches xT chunk 0; W2 chunk 1
                    # reads hT and writes chunk 1, so hide the apply per-di
                    # between W2 tiles (emitting it all up front starves the
                    # W2 stts behind it on the vector queue).
                    emit_w2(l, 1, st12=(st1c, st2c),
                            post=lambda t: ln_apply_di(
                                abt, bbt, gb["g2"], gb["h2"], 0, t,
                                x8out=(l + 1 < L)))
                    if l + 1 < L:
                        emit_v(l + 1, 0, 2)
                        emit_qk(l + 1, 0, (0,))
                        ln_rows_bcast(st1c, st2c, abt, bbt, 1)
                        emit_v(l + 1, 2, 4)
                        emit_qk(l + 1, 1, (0,))
                        emit_qk(l + 1, 2, (0,))
                        ln_apply(abt, bbt, gb["g2"], gb["h2"], 1, x8out=True)
                        emit_v(l + 1, 4, NT)
                        emit_qk(l + 1, 0, (1,))
                        emit_qk(l + 1, 1, (1,))
                        emit_qk(l + 1, 2, (1,))
                    else:
                        emit_head(0, 2)
                        ln_rows_bcast(st1c, st2c, abt, bbt, 1)
                        emit_head(2, 4)
                        ln_apply(abt, bbt, gb["g2"], gb["h2"], 1)
                        emit_head(4, NT)

    _split_multi_waits(nc)
    return nc


def _host_prep(inputs):
    """Pre-transpose / quantize weights, fold pos into X. Layout work."""
    import ml_dtypes
    bf = ml_dtypes.bfloat16
    e4 = ml_dtypes.float8_e4m3
    f32 = np.float32
    inp = {k: np.asarray(v, dtype=f32) for k, v in inputs.items()}

    pos = np.arange(S, dtype=f32)[:, None]
    div = np.exp(np.arange(0, D, 2, dtype=f32) * (-np.log(10000.0) / D)).astype(f32)
    pe = np.zeros((S, D), f32)
    pe[:, 0::2] = np.sin(pos * div)
    pe[:, 1::2] = np.cos(pos * div)

    # band mask: key tile partitions j'=0..127, query cols q'=0..158 (+pad)
    jj = np.arange(128)[:, None]
    qq = np.arange(160)[None, :]
    band01 = (((qq - jj) >= 0) & ((qq - jj) < WIN)).astype(f32)

    def colmajor(v):  # [L, X] -> [L, 128, X/128] col slices
        return np.ascontiguousarray(v.reshape(L, -1, 128).transpose(0, 2, 1))

    def lhsT_tiles(wT, n_out):
        # wT: [L, K, M] (w^T); -> [L, n_out, 128, K/128, 128] where
        # [l, t, p, ki, m] = wT[l, 128*ki + p, 128*t + m]
        Lw, Kw, Mw = wT.shape
        assert Mw == n_out * 128
        r = wT.reshape(Lw, Kw // 128, 128, n_out, 128)
        return np.ascontiguousarray(r.transpose(0, 3, 2, 1, 4))

    def wscale(w):
        rms = np.sqrt(np.mean(w.astype(np.float64) ** 2))
        return float(2.0 ** np.round(np.log2(2.0 / max(rms, 1e-30))))

    # fp8 DoubleRow tiles: pairs of K-tiles (2kt, 2kt+1) on free dim 1
    def qk_tiles8(wT, s):  # wT [D, D] -> [H, 128, KT2, 2, 128]
        r = (wT * s).reshape(KT2, 2, 128, H, 128)
        return np.ascontiguousarray(r.transpose(3, 2, 0, 1, 4)).astype(e4)

    def v_tiles8(wT, s):  # wT [D, D] -> [NSC, 128, KT2, 2, SC]
        r = (wT * s).reshape(KT2, 2, 128, NSC, SC)
        return np.ascontiguousarray(r.transpose(3, 2, 0, 1, 4)).astype(e4)

    wqk8 = np.empty((L, 2, H, 128, KT2, 2, 128), e4)
    wv8 = np.empty((L, NSC, 128, KT2, 2, SC), e4)
    dqqv = np.empty((L, 128, 3), f32)
    for l in range(L):
        sq = wscale(inp["Wq"][l])
        sk = wscale(inp["Wk"][l])
        sv = wscale(inp["Wv"][l])
        wqk8[l, 0] = qk_tiles8(inp["Wq"][l].T, sq)
        wqk8[l, 1] = qk_tiles8(inp["Wk"][l].T, sk)
        wv8[l] = v_tiles8(inp["Wv"][l].T, sv)
        dqqv[l, :, 0] = SCALE / sq
        dqqv[l, :, 1] = 1.0 / sk
        dqqv[l, :, 2] = 1.0 / sv

    skip_affine = bool(
        np.all(inp["ln1_g"] == 1.0) and np.all(inp["ln1_b"] == 0.0)
        and np.all(inp["ln2_g"] == 1.0) and np.all(inp["ln2_b"] == 0.0)
    )

    shared = {
        "band": band01.astype(bf),
        "ident": np.eye(128, dtype=f32).astype(bf),
        "wqk8": wqk8,
        "wv8": wv8,
        "dqqv": dqqv,
        "wot": lhsT_tiles(inp["Wo"].transpose(0, 2, 1), ND).astype(bf),
        "w1t": lhsT_tiles(inp["W1"].transpose(0, 2, 1), NF).astype(bf),
        "w2t": lhsT_tiles(inp["W2"].transpose(0, 2, 1), ND).astype(bf),
        "wht": np.ascontiguousarray(
            inp["Wh"].T.reshape(ND, 128, NOUT).transpose(1, 0, 2)).astype(bf),
        "bqs": colmajor(inp["bq"] * SCALE),
        "bkc": colmajor(inp["bk"]),
        "bvb": inp["bv"].reshape(L, 1, D).astype(bf),
        "boc": colmajor(inp["bo"]),
        "b1c": colmajor(inp["b1"]),
        "b2c": colmajor(inp["b2"]),
        "g1c": colmajor(inp["ln1_g"]),
        "h1c": colmajor(inp["ln1_b"]),
        "g2c": colmajor(inp["ln2_g"]),
        "h2c": colmajor(inp["ln2_b"]),
        "onesc": np.ones((128, 1), f32).astype(bf),
        "onesr": np.ones((1, 128), f32).astype(bf),
        "bhb": np.ascontiguousarray(inp["bh"].reshape(1, NOUT)),
    }
    in_maps = []
    for b in range(N_CORES):
        xb = (inp["X"][b] + pe).T  # [D, S]
        m = dict(shared)
        m["xt16"] = np.ascontiguousarray(xb).astype(bf)
        m["xt8"] = np.ascontiguousarray(
            xb.reshape(ND, 128, S).transpose(1, 0, 2)).astype(e4)
        in_maps.append(m)
    return in_maps, skip_affine


_NC_CACHE = {}


def run(inputs, trace=False, **spmd_kwargs):
    in_maps, skip_affine = _host_prep(inputs)
    key = ("nc", skip_affine)
    if key not in _NC_CACHE:
        _NC_CACHE[key] = build_program(skip_affine=skip_affine)
    nc = _NC_CACHE[key]
    res = run_bass_kernel_spmd(
        nc, in_maps, list(range(N_CORES)), trace=trace, **spmd_kwargs
    )
    out = np.concatenate([res.results[i]["out"] for i in range(N_CORES)], axis=0)
    return out, res


def kernel(**inputs) -> np.ndarray:
    out, _ = run(inputs, trace=False)
    return out
